# revision 1
# baseline (speedup 1.0000x reference)
"""DirGATConv on 8 Trainium2 NeuronCores (Bass/Tile).

Strategy (node/data parallel, no collectives):
  - Each core owns 6250 destination nodes (a contiguous range).
  - Phase A (replicated): compute h = x@W per direction for all nodes plus
    per-node attention projections es = x@(W a_src), ed = x@(W a_dst), and
    write gather tables to DRAM. Table row (320 fp32 = 1280 B):
      [h_head0(64) | 1.0 | h_head1 | 1.0 | h_head2 | 1.0 | h_head3 | 1.0 |
       es(4) | pad]
    The interleaved 1.0 columns make each head's mask-matmul accumulate the
    softmax denominator in the same PSUM tile as the numerator.
    Tables are split into two row-banks (25088 / 24960) because dma_gather
    indices are int16. A per-core local table holds ed for the core's own
    destinations (indices are dst - core*6250 < 6250).
  - Phase B: destinations are permuted into 51 blocks of <=128 nodes by a
    4-constraint bin-packing so each (block, direction, src-bank) needs at
    most 5 chunks of 128 edges. Per chunk: gather source rows, build 4
    per-head masks  maskp_h[e,d] = (dst_local[e]==d) * p[e,h]  with one fused
    tensor_scalar each, and matmul  maskp_h^T @ rows  into PSUM.
    p = exp(leaky_relu(es[src]+ed[dst])); softmax normalization is applied
    after aggregation (numerator and denominator are both linear in p).
"""

import numpy as np

import concourse.bacc as bacc
import concourse.mybir as mybir
import concourse.tile as tile
from concourse.bass_utils import run_bass_kernel_spmd
from concourse import library_config

# problem constants
N, E, DIN, H, C = 50000, 400000, 256, 4, 64
HC = H * C
ALPHA, SLOPE = 0.5, 0.2

# distribution constants
NCORES = 8
NPC = N // NCORES              # 6250 destinations per core
BANK0 = 25088                  # src-table bank split (196 tiles of 128)
BANK1 = N + 48 - BANK0         # 24960 (195 tiles); tables padded to 50048 rows
NT0, NT1 = BANK0 // 128, BANK1 // 128
NTILE = NT0 + NT1              # 391
NBIN = 51                      # destination blocks per core
CB = 5                         # chunks per (block, src-bank)
CPB = 2 * CB                   # chunks per block
NLOCT = 49                     # local tiles (49*128 = 6272 >= 6250)
NLOC = NLOCT * 128
TW = 320                       # table row width (floats); 1280 B
LW = 64                        # local table row width; 256 B
F32 = mybir.dt.float32
I16 = mybir.dt.int16


def build_kernel(num_swdge_queues=1, gather_queues=1, mode="full", nbin=NBIN):
    nc = bacc.Bacc("TRN2", num_swdge_queues=num_swdge_queues)

    x = nc.dram_tensor("x", [N, DIN], F32, kind="ExternalInput")
    x_loc = nc.dram_tensor("x_loc", [NLOC, DIN], F32, kind="ExternalInput")
    W1 = nc.dram_tensor("W1", [DIN, HC], F32, kind="ExternalInput")
    W2 = nc.dram_tensor("W2", [DIN, HC], F32, kind="ExternalInput")
    a_in = nc.dram_tensor("a_in", [4, H, C, 1], F32, kind="ExternalInput")  # src1,dst1,src2,dst2
    b_in = nc.dram_tensor("b_in", [1, HC], F32, kind="ExternalInput")    # 0.5*(b1+b2)
    iota_in = nc.dram_tensor("iota_in", [128, 128], F32, kind="ExternalInput")
    ident_in = nc.dram_tensor("ident_in", [128, 128], F32, kind="ExternalInput")
    gidx = nc.dram_tensor("gidx", [2, NBIN, 128, 160], I16, kind="ExternalInput")
    dcol = nc.dram_tensor("dcol", [2, NBIN, 128, CPB], F32, kind="ExternalInput")
    out = nc.dram_tensor("out", [NBIN * 128, HC], F32, kind="ExternalOutput")

    with tile.TileContext(nc) as tc:
        with (
            tc.tile_pool(name="dram", bufs=1, space="DRAM") as dpool,
            tc.tile_pool(name="const", bufs=1) as cpool,
        ):
            nc.gpsimd.load_library(library_config.mlp)

            tabs = [
                [dpool.tile([BANK0, TW], F32, tag=f"tab{d}0", name=f"tab{d}0"),
                 dpool.tile([BANK1, TW], F32, tag=f"tab{d}1", name=f"tab{d}1")]
                for d in range(2)
            ]
            loctab = dpool.tile([NLOC, LW], F32, tag="loctab")

            iota_t = cpool.tile([128, 128], F32)
            nc.sync.dma_start(iota_t[:], iota_in[:])
            ident_t = cpool.tile([128, 128], F32)
            nc.sync.dma_start(ident_t[:], ident_in[:])

            # weights: W_sb[d][k] = W_{d+1}[k*128:(k+1)*128, :]   [128 din, 256]
            W_sb = [[cpool.tile([128, HC], F32, tag=f"w{d}{k}", name=f"w{d}{k}")
                     for k in range(2)] for d in range(2)]
            for d, wdram in enumerate((W1, W2)):
                for k in range(2):
                    nc.sync.dma_start(W_sb[d][k][:], wdram[k * 128:(k + 1) * 128, :])

            # A matrices: A[d][kc]  [128 hc, 8]  block-diag of (a_src_d | a_dst_d)
            A_sb = [cpool.tile([128, 2, 8], F32, tag=f"a{d}", name=f"a{d}")
                    for d in range(2)]
            for d in range(2):
                nc.vector.memset(A_sb[d][:], 0.0)
                for j in range(2):          # 0: a_src, 1: a_dst
                    for h in range(H):
                        hc0 = h * C
                        kc, off = divmod(hc0, 128)
                        jj = j * H + h
                        nc.sync.dma_start(
                            A_sb[d][off:off + C, kc, jj:jj + 1],
                            a_in[2 * d + j, h, :, :],
                        )

            # WT[d][kc]  [128 hc, 256 din]  via PE transposes
            WT = [cpool.tile([128, 2, 256], F32, tag=f"wt{d}", name=f"wt{d}")
                  for d in range(2)]
            with tc.tile_pool(name="psA0", bufs=2, space="PSUM") as ps0:
                for d in range(2):
                    for kc in range(2):
                        for m in range(2):
                            pt = ps0.tile([128, 128], F32, tag="ptr")
                            nc.tensor.transpose(
                                pt[:], W_sb[d][m][:, kc * 128:(kc + 1) * 128],
                                ident_t[:])
                            nc.vector.tensor_copy(
                                WT[d][:, kc, m * 128:(m + 1) * 128], pt[:])
                # Wsd[k]  [128 din, 16]: cols 0:8 dir1 (es|ed), 8:16 dir2
                wsd = cpool.tile([128, 2, 16], F32)
                for m in range(2):
                    pw = ps0.tile([128, 16], F32, tag="pw")
                    for d in range(2):
                        for kc in range(2):
                            nc.tensor.matmul(
                                pw[:, d * 8:(d + 1) * 8],
                                WT[d][:, kc, m * 128:(m + 1) * 128],
                                A_sb[d][:, kc, :],
                                start=(kc == 0), stop=(kc == 1),
                            )
                    nc.vector.tensor_copy(wsd[:, m, :], pw[:])

                # bias broadcast tile: ones(128,1) x b_in(1,256)
                ones_row = cpool.tile([1, 128], F32)
                nc.vector.memset(ones_row[:], 1.0)
                brow = cpool.tile([1, HC], F32)
                nc.sync.dma_start(brow[:], b_in[:])
                bias_bc = cpool.tile([128, HC], F32)
                pb = ps0.tile([128, HC], F32, tag="pb")
                nc.tensor.matmul(pb[:], ones_row[:], brow[:])
                nc.vector.tensor_copy(bias_bc[:], pb[:])

            # ---------------- Phase A ----------------
            with (
                tc.tile_pool(name="pA", bufs=3) as pa,
                tc.tile_pool(name="pAst", bufs=3) as past,
                tc.tile_pool(name="psA", bufs=2, space="PSUM") as psa,
            ):
                def node_tile(xa_src, write_tabs, write_loc, partial):
                    xt = pa.tile([128, DIN], F32, tag="xt")
                    if partial:
                        nc.vector.memset(xt[:, :], 0.0)
                        nc.sync.dma_start(xt[:partial, :], xa_src)
                    else:
                        nc.sync.dma_start(xt[:], xa_src)
                    pxt = psa.tile([128, 2, 128], F32, tag="pxt")
                    for k in range(2):
                        nc.tensor.transpose(
                            pxt[:, k, :], xt[:, k * 128:(k + 1) * 128], ident_t[:])
                    xts = pa.tile([128, 2, 128], F32, tag="xts")
                    nc.vector.tensor_copy(xts[:], pxt[:])

                    pes = psa.tile([128, 16], F32, tag="pes")
                    for k in range(2):
                        nc.tensor.matmul(pes[:], xts[:, k, :], wsd[:, k, :],
                                         start=(k == 0), stop=(k == 1))
                    if write_tabs is not None:
                        for d in range(2):
                            ph = psa.tile([128, HC], F32, tag=f"ph{d}")
                            for k in range(2):
                                nc.tensor.matmul(
                                    ph[:], xts[:, k, :], W_sb[d][k][:],
                                    start=(k == 0), stop=(k == 1))
                            st = past.tile([128, TW], F32, tag=f"st{d}")
                            stv = st[:, 0:260].rearrange("p (h w) -> p h w", w=65)
                            nc.vector.tensor_copy(
                                stv[:, :, 0:64],
                                ph[:].rearrange("p (h w) -> p h w", w=64))
                            nc.vector.memset(stv[:, :, 64], 1.0)
                            nc.vector.tensor_copy(st[:, 260:264], pes[:, d * 8:d * 8 + 4])
                            nc.vector.memset(st[:, 264:TW], 0.0)
                            nc.sync.dma_start(write_tabs[d], st[:])
                    if write_loc is not None:
                        stl = past.tile([128, LW], F32, tag="stl")
                        nc.vector.tensor_copy(stl[:, 0:4], pes[:, 4:8])
                        nc.vector.tensor_copy(stl[:, 4:8], pes[:, 12:16])
                        nc.vector.memset(stl[:, 8:LW], 0.0)
                        nc.sync.dma_start(write_loc, stl[:])

                for t in range(NTILE):
                    partial = 80 if t == NTILE - 1 else 0
                    rows = x[t * 128: min((t + 1) * 128, N), :]
                    bk = 0 if t < NT0 else 1
                    r0 = t * 128 - (0 if bk == 0 else BANK0)
                    wt = [tabs[d][bk][r0:r0 + 128, :] for d in range(2)]
                    node_tile(rows, wt, None, partial)
                for t in range(NLOCT):
                    rows = x_loc[t * 128:(t + 1) * 128, :]
                    node_tile(rows, None, loctab[t * 128:(t + 1) * 128, :], 0)

            # ---------------- Phase B ----------------
            with (
                tc.tile_pool(name="pBg", bufs=2) as pg,
                tc.tile_pool(name="pBm", bufs=3) as pm,
                tc.tile_pool(name="pBmask", bufs=8) as pmask,
                tc.tile_pool(name="pBo", bufs=2) as po,
                tc.tile_pool(name="psB", bufs=4, space="PSUM") as psb,
            ):
                for b in range(0 if mode == "A" else nbin):
                    stage = [None, None]
                    for d in range(2):
                        gi = pm.tile([128, 160], I16, tag="gi")
                        nc.sync.dma_start(gi[:], gidx[d, b, :, :])
                        dc = pm.tile([128, CPB], F32, tag="dc")
                        nc.sync.dma_start(dc[:], dcol[d, b, :, :])

                        srcg = pg.tile([128, CPB, TW], F32, tag="srcg")
                        qq = (2 * b + d) % gather_queues
                        nc.gpsimd.dma_gather(
                            srcg[:, 0:CB, :], tabs[d][0][:], gi[:, 0:40],
                            CB * 128, CB * 128, TW, queue_num=qq)
                        nc.gpsimd.dma_gather(
                            srcg[:, CB:CPB, :], tabs[d][1][:], gi[:, 40:80],
                            CB * 128, CB * 128, TW, queue_num=qq)
                        tailg = pg.tile([128, CPB, LW], F32, tag="tailg")
                        nc.gpsimd.dma_gather(
                            tailg[:, 0:CB, :], loctab[:], gi[:, 80:120],
                            CB * 128, CB * 128, LW, queue_num=qq)
                        nc.gpsimd.dma_gather(
                            tailg[:, CB:CPB, :], loctab[:], gi[:, 120:160],
                            CB * 128, CB * 128, LW, queue_num=qq)

                        if mode == "G":
                            if d == 0:
                                nc.sync.dma_start(
                                    out[b * 128:(b + 1) * 128, :],
                                    srcg[:, 0, 0:HC])
                            continue

                        # p = exp(leaky_relu(es_src + ed_dst))  [128, CPB, 4]
                        lg = pm.tile([128, CPB, 4], F32, tag="lg")
                        nc.vector.tensor_tensor(
                            lg[:], srcg[:, :, 260:264],
                            tailg[:, :, 4 * d:4 * d + 4], mybir.AluOpType.add)
                        ls = pm.tile([128, CPB, 4], F32, tag="ls")
                        nc.vector.tensor_scalar(
                            out=ls[:], in0=lg[:], scalar1=SLOPE, scalar2=None,
                            op0=mybir.AluOpType.mult)
                        nc.vector.tensor_tensor(lg[:], lg[:], ls[:],
                                                mybir.AluOpType.max)
                        pt = pm.tile([128, CPB, 4], F32, tag="pt")
                        nc.scalar.activation(pt[:], lg[:],
                                             mybir.ActivationFunctionType.Exp)

                        pnd = psb.tile([128, H, 65], F32, tag="pnd")
                        for h in range(H):
                            for cc in range(CPB):
                                mp = pmask.tile([128, 128], F32, tag="mp")
                                nc.vector.tensor_scalar(
                                    out=mp[:], in0=iota_t[:],
                                    scalar1=dc[:, cc:cc + 1],
                                    scalar2=pt[:, cc, h:h + 1],
                                    op0=mybir.AluOpType.is_equal,
                                    op1=mybir.AluOpType.mult)
                                nc.tensor.matmul(
                                    pnd[:, h, :], mp[:], srcg[:, cc, 65 * h:65 * h + 65],
                                    start=(cc == 0), stop=(cc == CPB - 1))

                        # stage_d[:, h*64:(h+1)*64] = num_h / (2*den_h)
                        d2 = pm.tile([128, H], F32, tag="d2")
                        nc.vector.tensor_scalar(
                            out=d2[:], in0=pnd[:, :, 64], scalar1=2.0, scalar2=1e-9,
                            op0=mybir.AluOpType.mult, op1=mybir.AluOpType.max)
                        rec = pm.tile([128, H], F32, tag="rec")
                        nc.vector.reciprocal(rec[:], d2[:])
                        stage[d] = po.tile([128, HC], F32, tag=f"stage{d}", name=f"stage{d}")
                        for h in range(H):
                            nc.vector.tensor_scalar(
                                out=stage[d][:, h * 64:(h + 1) * 64],
                                in0=pnd[:, h, 0:64],
                                scalar1=rec[:, h:h + 1], scalar2=None,
                                op0=mybir.AluOpType.mult)

                    if mode == "G":
                        continue
                    ot = po.tile([128, HC], F32, tag="ot")
                    nc.vector.tensor_tensor(ot[:], stage[0][:], stage[1][:],
                                            mybir.AluOpType.add)
                    nc.vector.tensor_tensor(ot[:], ot[:], bias_bc[:],
                                            mybir.AluOpType.add)
                    nc.sync.dma_start(out[b * 128:(b + 1) * 128, :], ot[:])

    nc.compile()
    return nc


# ---------------------------------------------------------------- host side

def _wrap16(arr):
    """int idx array [n] -> dma_gather layout [128, n/16] int16 (replicated)."""
    n = len(arr)
    m = arr.reshape(n // 16, 16).astype(np.int16).T  # [16, n/16]
    return np.tile(m, (8, 1))


def prep_inputs(x, edge_index, W1, a_src1, a_dst1, b1, W2, a_src2, a_dst2, b2):
    x = np.asarray(x, np.float32)
    ei = np.asarray(edge_index)
    src, dst = ei[0].astype(np.int64), ei[1].astype(np.int64)
    loops = np.arange(N, dtype=np.int64)
    dirs = [
        (np.concatenate([src, loops]), np.concatenate([dst, loops])),
        (np.concatenate([dst, loops]), np.concatenate([src, loops])),
    ]

    # per-node degree by (dir, src-bank)
    deg = np.zeros((N, 4), np.int64)
    for j, (ss, dd) in enumerate(dirs):
        for bk in range(2):
            m = (ss >= BANK0) == (bk == 1)
            deg[:, 2 * j + bk] = np.bincount(dd[m], minlength=N)

    iota_in = np.broadcast_to(np.arange(128, dtype=np.float32), (128, 128)).copy()
    ident_in = np.eye(128, dtype=np.float32)
    a_in = np.stack([a_src1, a_dst1, a_src2, a_dst2]).astype(np.float32)[..., None]
    b_in = (0.5 * (np.asarray(b1) + np.asarray(b2))).astype(np.float32).reshape(1, HC)

    in_maps, perms = [], []
    for core in range(NCORES):
        lo = core * NPC
        nodes = np.arange(lo, lo + NPC)
        order = nodes[np.argsort(-deg[nodes].sum(1), kind="stable")]
        degs = deg[order]
        bins_load = np.zeros((NBIN, 4), np.int64)
        bins_cnt = np.zeros(NBIN, np.int64)
        node_blk = np.full(N, -1, np.int64)
        node_slot = np.full(N, -1, np.int64)
        for i_n in range(len(order)):
            dgl = degs[i_n]
            ok = (bins_cnt < 128) & ((bins_load + dgl) <= CB * 128).all(1)
            assert ok.any(), "bin packing failed; raise NBIN/CB"
            cand = np.where(ok)[0]
            nl = (bins_load[cand] + dgl).max(1) * 1000 + bins_cnt[cand]
            i = cand[np.argmin(nl)]
            node_blk[order[i_n]] = i
            node_slot[order[i_n]] = bins_cnt[i]
            bins_load[i] += dgl
            bins_cnt[i] += 1

        perm = np.full(NBIN * 128, -1, np.int64)
        perm[node_blk[nodes] * 128 + node_slot[nodes]] = nodes
        perms.append(perm)

        g_idx = np.zeros((2, NBIN, 128, 160), np.int16)
        d_col = np.full((2, NBIN, 128, CPB), -1.0, np.float32)
        for d, (ss, dd) in enumerate(dirs):
            sel = (dd >= lo) & (dd < lo + NPC)
            es_, ed_ = ss[sel], dd[sel]
            blk = node_blk[ed_]
            bank = (es_ >= BANK0).astype(np.int64)
            eo = np.lexsort((bank, blk))
            es_, ed_, blk, bank = es_[eo], ed_[eo], blk[eo], bank[eo]
            # slot position within (blk, bank) segment
            seg = blk * 2 + bank
            segbnd = np.flatnonzero(np.diff(seg, prepend=-1))
            within = np.arange(len(seg)) - np.repeat(segbnd, np.diff(
                np.append(segbnd, len(seg))))
            assert (within < CB * 128).all()
            slot = within + np.where(bank == 0, 0, CB * 128)
            srcrel = np.where(bank == 0, es_, es_ - BANK0)
            s_idx = np.zeros((NBIN, CPB * 128), np.int64)
            t_idx = np.zeros((NBIN, CPB * 128), np.int64)
            dloc = np.full((NBIN, CPB * 128), -1.0, np.float32)
            s_idx[blk, slot] = srcrel
            t_idx[blk, slot] = ed_ - lo
            dloc[blk, slot] = node_slot[ed_]
            for b in range(NBIN):
                g_idx[d, b, :, 0:40] = _wrap16(s_idx[b, 0:CB * 128])
                g_idx[d, b, :, 40:80] = _wrap16(s_idx[b, CB * 128:])
                g_idx[d, b, :, 80:160] = _wrap16(t_idx[b])
                d_col[d, b] = dloc[b].reshape(CPB, 128).T

        x_loc = np.zeros((NLOC, DIN), np.float32)
        x_loc[:NPC] = x[lo:lo + NPC]
        in_maps.append({
            "x": x, "x_loc": x_loc,
            "W1": np.asarray(W1, np.float32), "W2": np.asarray(W2, np.float32),
            "a_in": a_in, "b_in": b_in,
            "iota_in": iota_in, "ident_in": ident_in,
            "gidx": g_idx, "dcol": d_col,
        })
    return in_maps, perms


_NC_CACHE = {}


def kernel(**inputs):
    in_maps, perms = prep_inputs(**inputs)
    key = "k1"
    if key not in _NC_CACHE:
        _NC_CACHE[key] = build_kernel()
    nc = _NC_CACHE[key]
    res = run_bass_kernel_spmd(nc, in_maps, list(range(NCORES)))
    result = np.empty((N, HC), np.float32)
    for core in range(NCORES):
        o = res.results[core]["out"]
        p = perms[core]
        valid = p >= 0
        result[p[valid]] = o[valid]
    return result



# revision 13
# speedup vs baseline: 1.8505x; 1.8505x over previous
"""DirGATConv on 8 Trainium2 NeuronCores (Bass/Tile), v2.

Strategy (node/data parallel, no collectives):
  - Each core owns 6250 destination nodes, permuted into NBIN blocks of <=128
    by bin packing so every (block, direction, src-bank) has at most CB*128
    non-self-loop edges.
  - Phase A (replicated on every core): h = x @ W_d for all nodes plus the
    per-node attention projections es/ed = x @ (W_d a_*), written to two DRAM
    gather tables per direction (fp16 rows: 256 h | 4 es | 124 pad = 768 B;
    row count per bank <= 32767 because dma_gather indices are int16).  A
    bin-permuted local table per direction holds (h | es | ed) for the core's
    own destinations (544 B rows, read linearly in Phase B).
  - Phase B per (block, direction): dma_gather the source rows (one gather
    per src-bank), then with host-shipped 0/1 fp16 masks M [e,d] / MT [d,e]:
      ed_bc  = MT^T @ ed_tile                    (per-edge dst projection)
      p      = exp(lrelu(es + ed_bc) - ln 64)    (scalar engine; -ln64 keeps
                                                  h*p inside fp16 range)
      rows  *= p (per-head broadcast multiply), then one matmul per chunk
      num    = M^T @ rows, den = M^T @ p         (same stationary mask)
      out_d  = (num + p_self*h_loc) / (2*(den + p_self))
    Softmax normalization is exact because num and den are linear in p and
    any per-edge common factor (the -ln64 bias) cancels in num/den.
  - Host work is graph-structure-only (bin packing, gather indices, masks,
    layout transposes) plus standard weight fusion (W @ a projections).
"""

import numpy as np

import concourse.bacc as bacc
import concourse.mybir as mybir
import concourse.tile as tile
from concourse.bass_utils import run_bass_kernel_spmd
from concourse import library_config

# problem constants
N, E, DIN, H, C = 50000, 400000, 256, 4, 64
HC = H * C
ALPHA, SLOPE = 0.5, 0.2

# distribution constants
NCORES = 8
NPC = N // NCORES              # 6250 destinations per core
B0REAL = 24960                 # real nodes in bank 0 (nodes 0..24959)
BKROWS = 25088                 # rows per table bank (includes zero pad rows)
PADIDX = BKROWS - 1            # gather index for empty edge slots (zero row)
NT = 392                       # main node tiles (2 banks x 196)
NBIN = 51                      # destination blocks per core
CB = 5                         # gather chunks per (block, src-bank)
CPB = 2 * CB                   # chunks per block
NLOC = NBIN * 128
TW = 384                       # table row width (fp16) = 768 B
LW = 272                       # local row width (fp16) = 544 B
LNB = float(np.log(64.0))      # exp bias, cancels in num/den
F16 = mybir.dt.float16
F32 = mybir.dt.float32
I16 = mybir.dt.int16
AF = mybir.ActivationFunctionType
OP = mybir.AluOpType


def build_kernel(nbin=NBIN, cb=CB, debug=False):
    cpb = 2 * cb
    nc = bacc.Bacc("TRN2", num_swdge_queues=2)
    if debug:
        dbg_srcg = nc.dram_tensor("dbg_srcg", [128, cpb * TW], F16, kind="ExternalOutput")
        dbg_ped = nc.dram_tensor("dbg_ped", [128, cpb * 4], F32, kind="ExternalOutput")
        dbg_p = nc.dram_tensor("dbg_p", [128, cpb * 4], F16, kind="ExternalOutput")
        dbg_pnd = nc.dram_tensor("dbg_pnd", [128, HC + 4], F32, kind="ExternalOutput")
        dbg_lc = nc.dram_tensor("dbg_lc", [128, LW], F16, kind="ExternalOutput")
        dbg_srcg2 = nc.dram_tensor("dbg_srcg2", [128, cpb * TW], F16, kind="ExternalOutput")

    xTb = nc.dram_tensor("xTb", [2, 128, NT * 128], F16, kind="ExternalInput")
    xTl = nc.dram_tensor("xTl", [2, 128, nbin * 128], F16, kind="ExternalInput")
    Wsb = nc.dram_tensor("Wsb", [2, 2, 128, HC], F16, kind="ExternalInput")
    wsd_in = nc.dram_tensor("wsd_in", [2, 128, 16], F16, kind="ExternalInput")
    b_in = nc.dram_tensor("b_in", [1, HC], F32, kind="ExternalInput")
    gidx = nc.dram_tensor("gidx", [2, nbin, 128, 16 * cb], I16, kind="ExternalInput")
    msk = nc.dram_tensor("msk", [2, nbin, 2, 128, cpb * 128], F16, kind="ExternalInput")
    out = nc.dram_tensor("out", [nbin * 128, HC], F32, kind="ExternalOutput")

    with tile.TileContext(nc) as tc:
        with (
            tc.tile_pool(name="dram", bufs=1, space="DRAM") as dpool,
            tc.tile_pool(name="const", bufs=1) as cpool,
        ):
            nc.gpsimd.load_library(library_config.mlp)

            tabs = [
                [dpool.tile([BKROWS, TW], F16, tag=f"tab{d}{k}", name=f"tab{d}{k}")
                 for k in range(2)]
                for d in range(2)
            ]
            locs = [dpool.tile([nbin * 128, LW], F16, tag=f"loc{d}", name=f"loc{d}")
                    for d in range(2)]

            # weights: w_sb[d] [128 din, 2 k, 256 hc]
            w_sb = [cpool.tile([128, 2, HC], F16, tag=f"w{d}", name=f"w{d}")
                    for d in range(2)]
            for d in range(2):
                nc.sync.dma_start(
                    w_sb[d][:], Wsb[d].rearrange("k p c -> p k c"))
            wsd = cpool.tile([128, 2, 16], F16)
            nc.sync.dma_start(wsd[:], wsd_in.rearrange("k p c -> p k c"))
            bias_bc = cpool.tile([128, HC], F32)
            nc.sync.dma_start(bias_bc[:], b_in[:].to_broadcast((128, HC)))
            lnb_t = cpool.tile([128, 1], F32)
            nc.vector.memset(lnb_t[:], -LNB)

            # ---------------- Phase A ----------------
            with (
                tc.tile_pool(name="pAx", bufs=2) as pax,
                tc.tile_pool(name="pAs", bufs=2) as pas,
                tc.tile_pool(name="psA", bufs=2, space="PSUM") as psa,
            ):
                st = [None, None]

                def node_tile(xt_k, j, wide):
                    """One 128-node tile: xt_k [128, 2, 128]; write into
                    st[d][:, j, :] (wide=TW) or st[d] [128, LW] (wide=LW)."""
                    ph0 = psa.tile([128, HC], F32, tag="ph0")
                    ph1 = psa.tile([128, HC], F32, tag="ph1")
                    ph = [ph0, ph1]
                    pes = psa.tile([128, 16], F32, tag="pes")
                    for k in range(2):
                        for d in range(2):
                            nc.tensor.matmul(
                                ph[d][:], xt_k[:, k, :], w_sb[d][:, k, :],
                                start=(k == 0), stop=(k == 1))
                        nc.tensor.matmul(pes[:], xt_k[:, k, :], wsd[:, k, :],
                                         start=(k == 0), stop=(k == 1))
                    if wide == TW:
                        nc.vector.tensor_copy(st[0][:, j, 0:HC], ph[0][:])
                        nc.vector.tensor_copy(st[0][:, j, HC:HC + 4], pes[:, 0:4])
                        nc.scalar.activation(st[1][:, j, 0:HC], ph[1][:], AF.Copy)
                        nc.scalar.activation(st[1][:, j, HC:HC + 4], pes[:, 8:12],
                                             AF.Copy)
                    else:
                        nc.vector.tensor_copy(st[0][:, 0:HC], ph[0][:])
                        nc.vector.tensor_copy(st[0][:, HC:HC + 8], pes[:, 0:8])
                        nc.scalar.activation(st[1][:, 0:HC], ph[1][:], AF.Copy)
                        nc.scalar.activation(st[1][:, HC:HC + 8], pes[:, 8:16],
                                             AF.Copy)

                # main tiles: 4-tile batches (bank boundary at tile 196 = 49*4)
                for it in range(NT // 4):
                    xt = pax.tile([128, 4, 2, 128], F16, tag="xt")
                    for k in range(2):
                        nc.sync.dma_start(
                            xt[:, :, k, :],
                            xTb[k, :, it * 512:(it + 1) * 512].rearrange(
                                "p (t c) -> p t c", c=128))
                    for d in range(2):
                        st[d] = pas.tile([128, 4, TW], F16, tag=f"st{d}", name=f"st{d}")
                    for t in range(4):
                        node_tile(xt[:, t, :, :], t, TW)
                    t0 = it * 4
                    bk = 0 if t0 < 196 else 1
                    r0 = (t0 - (0 if bk == 0 else 196)) * 128
                    for d in range(2):
                        dst = tabs[d][bk][r0:r0 + 512, :].rearrange(
                            "(t p) c -> p t c", t=4)
                        eng = nc.gpsimd if d == 0 else nc.scalar
                        eng.dma_start(dst, st[d][:])

                # local tiles (one per iteration)
                for t in range(nbin):
                    xt = pax.tile([128, 1, 2, 128], F16, tag="xt")
                    for k in range(2):
                        nc.sync.dma_start(
                            xt[:, 0, k, :],
                            xTl[k, :, t * 128:(t + 1) * 128])
                    for d in range(2):
                        st[d] = pas.tile([128, LW], F16, tag=f"lst{d}", name=f"lst{d}")
                    node_tile(xt[:, 0, :, :], 0, LW)
                    for d in range(2):
                        eng = nc.gpsimd if d == 0 else nc.scalar
                        eng.dma_start(locs[d][t * 128:(t + 1) * 128, :], st[d][:])

            # ---------------- Phase B ----------------
            with (
                tc.tile_pool(name="pBg", bufs=3) as pg,
                tc.tile_pool(name="pBk", bufs=3) as pk,
                tc.tile_pool(name="pBm", bufs=3) as pm,
                tc.tile_pool(name="pBo", bufs=2) as po,
                tc.tile_pool(name="psN", bufs=2, space="PSUM") as psn,
                tc.tile_pool(name="psE", bufs=2, space="PSUM") as pse,
            ):
                for b in range(nbin):
                    stage = [None, None]
                    for d in range(2):
                        gi = pm.tile([128, 16 * cb], I16, tag="gi")
                        nc.sync.dma_start(gi[:], gidx[d, b])
                        mk = pk.tile([128, 2, cpb * 128], F16, tag="mk")
                        nc.sync.dma_start(mk[:], msk[d, b].rearrange("m p c -> p m c"))
                        lc = pm.tile([128, LW], F16, tag="lc")
                        nc.sync.dma_start(lc[:], locs[d][b * 128:(b + 1) * 128, :])

                        srcg = pg.tile([128, cpb, TW], F16, tag="srcg")
                        for half in range(2):
                            nc.gpsimd.dma_gather(
                                srcg[:, half * cb:(half + 1) * cb, :],
                                tabs[d][half][:],
                                gi[:, half * 8 * cb:(half + 1) * 8 * cb],
                                cb * 128, cb * 128, TW,
                                queue_num=(2 * b + d) % 2)

                        if debug and b == 0 and d == 0:
                            nc.sync.dma_start(dbg_srcg[:], srcg[:].rearrange("p a b -> p (a b)"))
                            nc.sync.dma_start(dbg_lc[:], lc[:])
                        # ed_bc[e, h] via MT^T @ ed_tile
                        ps_ed = pse.tile([128, cpb, 4], F32, tag="ped")
                        for c in range(cpb):
                            nc.tensor.matmul(
                                ps_ed[:, c, :], mk[:, 1, c * 128:(c + 1) * 128],
                                lc[:, HC + 4:HC + 8], start=True, stop=True)
                        # logits l = es + ed_bc ; p = exp(lrelu(l) - ln64)
                        lg = pm.tile([128, cpb, 4], F32, tag="lg")
                        nc.vector.tensor_tensor(
                            lg[:], srcg[:, :, HC:HC + 4], ps_ed[:], OP.add)
                        lr = pm.tile([128, cpb, 4], F32, tag="lr")
                        nc.scalar.activation(lr[:], lg[:], AF.Prelu, alpha=SLOPE)
                        pf = pm.tile([128, cpb, 4], F32, tag="pf")
                        nc.scalar.activation(pf[:], lr[:], AF.Exp, bias=lnb_t[:])
                        p16 = pm.tile([128, cpb, 4], F16, tag="p16")
                        nc.scalar.activation(p16[:], pf[:], AF.Copy)

                        if debug and b == 0 and d == 0:
                            nc.sync.dma_start(dbg_p[:], p16[:].rearrange("p a b -> p (a b)"))
                            ped_sb = pm.tile([128, cpb, 4], F32, tag="pedsb")
                            nc.vector.tensor_copy(ped_sb[:], ps_ed[:])
                            nc.sync.dma_start(dbg_ped[:], ped_sb[:].rearrange("p a b -> p (a b)"))
                        # rows *= p (per-head, per-partition scalar)
                        for c in range(cpb):
                            for h in range(H):
                                v = srcg[:, c, h * C:(h + 1) * C]
                                nc.vector.tensor_scalar(
                                    out=v, in0=v, scalar1=pf[:, c, h:h + 1],
                                    scalar2=None, op0=OP.mult)

                        if debug and b == 0 and d == 0:
                            nc.sync.dma_start(dbg_srcg2[:], srcg[:].rearrange("p a b -> p (a b)"))
                        # num/den accumulation
                        pnd = psn.tile([128, HC], F32, tag="pnd")
                        pden = pse.tile([128, 4], F32, tag="pden")
                        for c in range(cpb):
                            mc = mk[:, 0, c * 128:(c + 1) * 128]
                            nc.tensor.matmul(pnd[:, 0:HC], mc, srcg[:, c, 0:HC],
                                             start=(c == 0), stop=(c == cpb - 1))
                            nc.tensor.matmul(pden[:], mc, p16[:, c, :],
                                             start=(c == 0), stop=(c == cpb - 1))

                        if debug and b == 0 and d == 0:
                            pnd_sb = pm.tile([128, HC + 4], F32, tag="pndsb")
                            nc.vector.tensor_copy(pnd_sb[:, 0:HC], pnd[:])
                            nc.vector.tensor_copy(pnd_sb[:, HC:], pden[:])
                            nc.sync.dma_start(dbg_pnd[:], pnd_sb[:])
                        # self-loop p
                        sl = pm.tile([128, 4], F32, tag="sl")
                        nc.vector.tensor_tensor(
                            sl[:], lc[:, HC:HC + 4], lc[:, HC + 4:HC + 8], OP.add)
                        slr = pm.tile([128, 4], F32, tag="slr")
                        nc.scalar.activation(slr[:], sl[:], AF.Prelu, alpha=SLOPE)
                        psf = pm.tile([128, 4], F32, tag="psf")
                        nc.scalar.activation(psf[:], slr[:], AF.Exp, bias=lnb_t[:])

                        # normalize: stage = (num + p_self*h_loc) / (2*(den+p_self))
                        dtot = pm.tile([128, 4], F32, tag="dtot")
                        nc.vector.tensor_tensor(dtot[:], pden[:], psf[:],
                                                OP.add)
                        nc.vector.tensor_scalar(
                            out=dtot[:], in0=dtot[:], scalar1=2.0, scalar2=1e-30,
                            op0=OP.mult, op1=OP.max)
                        rec = pm.tile([128, 4], F32, tag="rec")
                        nc.vector.reciprocal(rec[:], dtot[:])

                        stg = po.tile([128, H, C], F32, tag=f"stg{d}", name=f"stg{d}")
                        for h in range(H):
                            nc.vector.tensor_scalar(
                                out=stg[:, h, :], in0=lc[:, h * C:(h + 1) * C],
                                scalar1=psf[:, h:h + 1], scalar2=None, op0=OP.mult)
                        nc.vector.tensor_tensor(
                            stg[:], stg[:],
                            pnd[:, 0:HC].rearrange("p (h w) -> p h w", w=C), OP.add)
                        for h in range(H):
                            nc.vector.tensor_scalar(
                                out=stg[:, h, :], in0=stg[:, h, :],
                                scalar1=rec[:, h:h + 1], scalar2=None, op0=OP.mult)
                        stage[d] = stg

                    ot = po.tile([128, HC], F32, tag="ot")
                    nc.vector.tensor_tensor(
                        ot[:].rearrange("p (h w) -> p h w", w=C),
                        stage[0][:], stage[1][:], OP.add)
                    nc.vector.tensor_tensor(ot[:], ot[:], bias_bc[:], OP.add)
                    nc.sync.dma_start(out[b * 128:(b + 1) * 128, :], ot[:])

    nc.compile()
    return nc


# ---------------------------------------------------------------- host side

def _wrap16(arr):
    """int idx array [n] -> dma_gather layout [128, n/16] int16 (replicated)."""
    n = len(arr)
    m = arr.reshape(n // 16, 16).astype(np.int16).T  # [16, n/16]
    return np.tile(m, (8, 1))


def prep_inputs(x, edge_index, W1, a_src1, a_dst1, b1, W2, a_src2, a_dst2, b2,
                nbin=NBIN, cb=CB):
    cpb = 2 * cb
    x = np.asarray(x, np.float32)
    ei = np.asarray(edge_index)
    src, dst = ei[0].astype(np.int64), ei[1].astype(np.int64)
    dirs = [(src, dst), (dst, src)]   # no self-loops; handled via local path

    # per-node degree by (dir, src-bank)
    deg = np.zeros((N, 4), np.int64)
    for j, (ss, dd) in enumerate(dirs):
        for bk in range(2):
            m = (ss >= B0REAL) == (bk == 1)
            deg[:, 2 * j + bk] = np.bincount(dd[m], minlength=N)

    # fp16 feature layouts
    x16 = x.astype(np.float16)
    xpad = np.zeros((NT * 128, DIN), np.float16)
    xpad[0:B0REAL] = x16[0:B0REAL]
    xpad[BKROWS:BKROWS + (N - B0REAL)] = x16[B0REAL:N]
    xTb = np.ascontiguousarray(xpad.T.reshape(2, 128, NT * 128))

    W_l = [np.asarray(W1, np.float32), np.asarray(W2, np.float32)]
    a_l = [(np.asarray(a_src1, np.float32), np.asarray(a_dst1, np.float32)),
           (np.asarray(a_src2, np.float32), np.asarray(a_dst2, np.float32))]
    Wsb = np.zeros((2, 2, 128, HC), np.float16)
    for d in range(2):
        for k in range(2):
            Wsb[d, k] = W_l[d][k * 128:(k + 1) * 128, :].astype(np.float16)
    cols = []
    for d in range(2):
        for a in a_l[d]:
            A = np.zeros((HC, H), np.float32)
            for h in range(H):
                A[h * C:(h + 1) * C, h] = a[h]
            cols.append(W_l[d] @ A)         # [256, 4]
    wsd_full = np.concatenate(cols, axis=1)  # [256, 16] (es1|ed1|es2|ed2)
    wsd_in = np.ascontiguousarray(
        wsd_full.reshape(2, 128, 16)).astype(np.float16)
    b_in = (0.5 * (np.asarray(b1) + np.asarray(b2))).astype(np.float32)
    b_in = b_in.reshape(1, HC)

    in_maps, perms = [], []
    for core in range(NCORES):
        lo = core * NPC
        nodes = np.arange(lo, lo + NPC)
        order = nodes[np.argsort(-deg[nodes].sum(1), kind="stable")]
        degs = deg[order]
        bins_load = np.zeros((nbin, 4), np.int64)
        bins_cnt = np.zeros(nbin, np.int64)
        node_blk = np.full(N, -1, np.int64)
        node_slot = np.full(N, -1, np.int64)
        for i_n in range(len(order)):
            dgl = degs[i_n]
            ok = (bins_cnt < 128) & ((bins_load + dgl) <= cb * 128).all(1)
            assert ok.any(), "bin packing failed; raise nbin/cb"
            cand = np.where(ok)[0]
            nl = (bins_load[cand] + dgl).max(1) * 1000 + bins_cnt[cand]
            i = cand[np.argmin(nl)]
            node_blk[order[i_n]] = i
            node_slot[order[i_n]] = bins_cnt[i]
            bins_load[i] += dgl
            bins_cnt[i] += 1

        perm = np.full(nbin * 128, -1, np.int64)
        perm[node_blk[nodes] * 128 + node_slot[nodes]] = nodes
        perms.append(perm)

        g_idx = np.zeros((2, nbin, 128, 16 * cb), np.int16)
        m_host = np.zeros((2, nbin, 2, 128, cpb * 128), np.float16)
        for d, (ss, dd) in enumerate(dirs):
            sel = (dd >= lo) & (dd < lo + NPC)
            es_, ed_ = ss[sel], dd[sel]
            blk = node_blk[ed_]
            bank = (es_ >= B0REAL).astype(np.int64)
            eo = np.lexsort((bank, blk))
            es_, ed_, blk, bank = es_[eo], ed_[eo], blk[eo], bank[eo]
            seg = blk * 2 + bank
            segbnd = np.flatnonzero(np.diff(seg, prepend=-1))
            within = np.arange(len(seg)) - np.repeat(segbnd, np.diff(
                np.append(segbnd, len(seg))))
            assert (within < cb * 128).all()
            slot = within + bank * (cb * 128)
            srcrel = np.where(bank == 0, es_, es_ - B0REAL)
            dslot = node_slot[ed_]
            s_idx = np.full((nbin, cpb * 128), PADIDX, np.int64)
            s_idx[blk, slot] = srcrel
            chunk, epart = slot // 128, slot % 128
            m_host[d, blk, 0, epart, chunk * 128 + dslot] = 1.0
            m_host[d, blk, 1, dslot, chunk * 128 + epart] = 1.0
            for bb in range(nbin):
                g_idx[d, bb, :, 0:8 * cb] = _wrap16(s_idx[bb, 0:cb * 128])
                g_idx[d, bb, :, 8 * cb:16 * cb] = _wrap16(s_idx[bb, cb * 128:])

        xloc = np.zeros((nbin * 128, DIN), np.float16)
        valid = perm >= 0
        xloc[valid] = x16[perm[valid]]
        xTl = np.ascontiguousarray(xloc.T.reshape(2, 128, nbin * 128))

        in_maps.append({
            "xTb": xTb, "xTl": xTl, "Wsb": Wsb, "wsd_in": wsd_in, "b_in": b_in,
            "gidx": g_idx, "msk": m_host,
        })
    return in_maps, perms


_NC_CACHE = {}


def kernel(**inputs):
    in_maps, perms = prep_inputs(**inputs)
    key = (NBIN, CB)
    if key not in _NC_CACHE:
        _NC_CACHE[key] = build_kernel(NBIN, CB)
    nc = _NC_CACHE[key]
    res = run_bass_kernel_spmd(nc, in_maps, list(range(NCORES)))
    result = np.empty((N, HC), np.float32)
    for core in range(NCORES):
        o = res.results[core]["out"]
        p = perms[core]
        valid = p >= 0
        result[p[valid]] = o[valid]
    return result


# revision 14
# speedup vs baseline: 2.1723x; 1.1739x over previous
"""DirGATConv on 8 Trainium2 NeuronCores (Bass/Tile), v2.

Strategy (node/data parallel, no collectives):
  - Each core owns 6250 destination nodes, permuted into NBIN blocks of <=128
    by bin packing so every (block, direction, src-bank) has at most CB*128
    non-self-loop edges.
  - Phase A (replicated on every core): h = x @ W_d for all nodes plus the
    per-node attention projections es/ed = x @ (W_d a_*), written to two DRAM
    gather tables per direction (fp16 rows: 256 h | 4 es | 124 pad = 768 B;
    row count per bank <= 32767 because dma_gather indices are int16).  A
    bin-permuted local table per direction holds (h | es | ed) for the core's
    own destinations (544 B rows, read linearly in Phase B).
  - Phase B per (block, direction): dma_gather the source rows (one gather
    per src-bank), then with host-shipped 0/1 fp16 masks M [e,d] / MT [d,e]:
      ed_bc  = MT^T @ ed_tile                    (per-edge dst projection)
      p      = exp(lrelu(es + ed_bc) - ln 64)    (scalar engine; -ln64 keeps
                                                  h*p inside fp16 range)
      rows  *= p (per-head broadcast multiply), then one matmul per chunk
      num    = M^T @ rows, den = M^T @ p         (same stationary mask)
      out_d  = (num + p_self*h_loc) / (2*(den + p_self))
    Softmax normalization is exact because num and den are linear in p and
    any per-edge common factor (the -ln64 bias) cancels in num/den.
  - Host work is graph-structure-only (bin packing, gather indices, masks,
    layout transposes) plus standard weight fusion (W @ a projections).
"""

import numpy as np

import concourse.bacc as bacc
import concourse.mybir as mybir
import concourse.tile as tile
from concourse.bass_utils import run_bass_kernel_spmd
from concourse import library_config

# problem constants
N, E, DIN, H, C = 50000, 400000, 256, 4, 64
HC = H * C
ALPHA, SLOPE = 0.5, 0.2

# distribution constants
NCORES = 8
NPC = N // NCORES              # 6250 destinations per core
B0REAL = 24960                 # real nodes in bank 0 (nodes 0..24959)
BKROWS = 25088                 # rows per table bank (includes zero pad rows)
PADIDX = BKROWS - 1            # gather index for empty edge slots (zero row)
NT = 392                       # main node tiles (2 banks x 196)
NBIN = 51                      # destination blocks per core
CB = 5                         # gather chunks per (block, src-bank)
CPB = 2 * CB                   # chunks per block
NLOC = NBIN * 128
TW = 384                       # table row width (fp16) = 768 B
LW = 272                       # local row width (fp16) = 544 B
LNB = float(np.log(64.0))      # exp bias, cancels in num/den
F16 = mybir.dt.float16
F32 = mybir.dt.float32
I16 = mybir.dt.int16
AF = mybir.ActivationFunctionType
OP = mybir.AluOpType


def build_kernel(nbin=NBIN, cb=CB, debug=False):
    cpb = 2 * cb
    nc = bacc.Bacc("TRN2", num_swdge_queues=2)
    if debug:
        dbg_srcg = nc.dram_tensor("dbg_srcg", [128, cpb * TW], F16, kind="ExternalOutput")
        dbg_ped = nc.dram_tensor("dbg_ped", [128, cpb * 4], F32, kind="ExternalOutput")
        dbg_p = nc.dram_tensor("dbg_p", [128, cpb * 4], F16, kind="ExternalOutput")
        dbg_pnd = nc.dram_tensor("dbg_pnd", [128, HC + 4], F32, kind="ExternalOutput")
        dbg_lc = nc.dram_tensor("dbg_lc", [128, LW], F16, kind="ExternalOutput")
        dbg_srcg2 = nc.dram_tensor("dbg_srcg2", [128, cpb * TW], F16, kind="ExternalOutput")

    xTb = nc.dram_tensor("xTb", [2, 128, NT * 128], F16, kind="ExternalInput")
    xTl = nc.dram_tensor("xTl", [2, 128, nbin * 128], F16, kind="ExternalInput")
    Wsb = nc.dram_tensor("Wsb", [2, 2, 128, HC], F16, kind="ExternalInput")
    wsd_in = nc.dram_tensor("wsd_in", [2, 128, 16], F16, kind="ExternalInput")
    b_in = nc.dram_tensor("b_in", [1, HC], F32, kind="ExternalInput")
    gidx = nc.dram_tensor("gidx", [2, nbin, 128, 16 * cb], I16, kind="ExternalInput")
    msk = nc.dram_tensor("msk", [2, nbin, 2, 128, cpb * 128], F16, kind="ExternalInput")
    out = nc.dram_tensor("out", [nbin * 128, HC], F32, kind="ExternalOutput")

    with tile.TileContext(nc) as tc:
        with (
            tc.tile_pool(name="dram", bufs=1, space="DRAM") as dpool,
            tc.tile_pool(name="const", bufs=1) as cpool,
        ):
            nc.gpsimd.load_library(library_config.mlp)

            tabs = [
                [dpool.tile([BKROWS, TW], F16, tag=f"tab{d}{k}", name=f"tab{d}{k}")
                 for k in range(2)]
                for d in range(2)
            ]
            locs = [dpool.tile([nbin * 128, LW], F16, tag=f"loc{d}", name=f"loc{d}")
                    for d in range(2)]

            # weights: w_sb[d] [128 din, 2 k, 256 hc]
            w_sb = [cpool.tile([128, 2, HC], F16, tag=f"w{d}", name=f"w{d}")
                    for d in range(2)]
            for d in range(2):
                nc.sync.dma_start(
                    w_sb[d][:], Wsb[d].rearrange("k p c -> p k c"))
            wsd = cpool.tile([128, 2, 16], F16)
            nc.sync.dma_start(wsd[:], wsd_in.rearrange("k p c -> p k c"))
            bias_bc = cpool.tile([128, HC], F32)
            nc.sync.dma_start(bias_bc[:], b_in[:].to_broadcast((128, HC)))
            lnb_t = cpool.tile([128, 1], F32)
            nc.vector.memset(lnb_t[:], -LNB)

            # ---------------- Phase A ----------------
            with (
                tc.tile_pool(name="pAx", bufs=2) as pax,
                tc.tile_pool(name="pAs", bufs=2) as pas,
                tc.tile_pool(name="psA", bufs=2, space="PSUM") as psa,
            ):
                st = [None, None]

                def node_tile(xt_k, j, wide):
                    """One 128-node tile: xt_k [128, 2, 128]; write into
                    st[d][:, j, :] (wide=TW) or st[d] [128, LW] (wide=LW)."""
                    ph0 = psa.tile([128, HC], F32, tag="ph0")
                    ph1 = psa.tile([128, HC], F32, tag="ph1")
                    ph = [ph0, ph1]
                    pes = psa.tile([128, 16], F32, tag="pes")
                    for k in range(2):
                        for d in range(2):
                            nc.tensor.matmul(
                                ph[d][:], xt_k[:, k, :], w_sb[d][:, k, :],
                                start=(k == 0), stop=(k == 1))
                        nc.tensor.matmul(pes[:], xt_k[:, k, :], wsd[:, k, :],
                                         start=(k == 0), stop=(k == 1))
                    if wide == TW:
                        nc.vector.tensor_copy(st[0][:, j, 0:HC], ph[0][:])
                        nc.vector.tensor_copy(st[0][:, j, HC:HC + 4], pes[:, 0:4])
                        nc.scalar.activation(st[1][:, j, 0:HC], ph[1][:], AF.Copy)
                        nc.scalar.activation(st[1][:, j, HC:HC + 4], pes[:, 8:12],
                                             AF.Copy)
                    else:
                        nc.vector.tensor_copy(st[0][:, 0:HC], ph[0][:])
                        nc.vector.tensor_copy(st[0][:, HC:HC + 8], pes[:, 0:8])
                        nc.scalar.activation(st[1][:, 0:HC], ph[1][:], AF.Copy)
                        nc.scalar.activation(st[1][:, HC:HC + 8], pes[:, 8:16],
                                             AF.Copy)

                # main tiles: 4-tile batches (bank boundary at tile 196 = 49*4)
                for it in range(NT // 4):
                    xt = pax.tile([128, 4, 2, 128], F16, tag="xt")
                    for k in range(2):
                        nc.sync.dma_start(
                            xt[:, :, k, :],
                            xTb[k, :, it * 512:(it + 1) * 512].rearrange(
                                "p (t c) -> p t c", c=128))
                    for d in range(2):
                        st[d] = pas.tile([128, 4, TW], F16, tag=f"st{d}", name=f"st{d}")
                    for t in range(4):
                        node_tile(xt[:, t, :, :], t, TW)
                    t0 = it * 4
                    bk = 0 if t0 < 196 else 1
                    r0 = (t0 - (0 if bk == 0 else 196)) * 128
                    for d in range(2):
                        dst = tabs[d][bk][r0:r0 + 512, :].rearrange(
                            "(t p) c -> p t c", t=4)
                        eng = nc.gpsimd if d == 0 else nc.scalar
                        eng.dma_start(dst, st[d][:])

                # local tiles (one per iteration)
                for t in range(nbin):
                    xt = pax.tile([128, 1, 2, 128], F16, tag="xt")
                    for k in range(2):
                        nc.sync.dma_start(
                            xt[:, 0, k, :],
                            xTl[k, :, t * 128:(t + 1) * 128])
                    for d in range(2):
                        st[d] = pas.tile([128, LW], F16, tag=f"lst{d}", name=f"lst{d}")
                    node_tile(xt[:, 0, :, :], 0, LW)
                    for d in range(2):
                        eng = nc.gpsimd if d == 0 else nc.scalar
                        eng.dma_start(locs[d][t * 128:(t + 1) * 128, :], st[d][:])

            # ---------------- Phase B ----------------
            with (
                tc.tile_pool(name="pBg", bufs=3) as pg,
                tc.tile_pool(name="pBk", bufs=3) as pk,
                tc.tile_pool(name="pBm", bufs=3) as pm,
                tc.tile_pool(name="pBo", bufs=2) as po,
                tc.tile_pool(name="psN", bufs=2, space="PSUM") as psn,
                tc.tile_pool(name="psE", bufs=2, space="PSUM") as pse,
            ):
                for b in range(nbin):
                    stage = [None, None]
                    for d in range(2):
                        gi = pm.tile([128, 16 * cb], I16, tag="gi")
                        nc.sync.dma_start(gi[:], gidx[d, b])
                        mk = pk.tile([128, 2, cpb * 128], F16, tag="mk")
                        nc.sync.dma_start(mk[:], msk[d, b].rearrange("m p c -> p m c"))
                        lc = pm.tile([128, LW], F16, tag="lc")
                        nc.sync.dma_start(lc[:], locs[d][b * 128:(b + 1) * 128, :])

                        srcg = pg.tile([128, cpb, TW], F16, tag="srcg")
                        for half in range(2):
                            nc.gpsimd.dma_gather(
                                srcg[:, half * cb:(half + 1) * cb, :],
                                tabs[d][half][:],
                                gi[:, half * 8 * cb:(half + 1) * 8 * cb],
                                cb * 128, cb * 128, TW,
                                queue_num=(2 * b + d) % 2)

                        if debug and b == 0 and d == 0:
                            nc.sync.dma_start(dbg_srcg[:], srcg[:].rearrange("p a b -> p (a b)"))
                            nc.sync.dma_start(dbg_lc[:], lc[:])
                        # ed_bc[e, h] via MT^T @ ed_tile
                        ps_ed = pse.tile([128, cpb, 4], F32, tag="ped")
                        for c in range(cpb):
                            nc.tensor.matmul(
                                ps_ed[:, c, :], mk[:, 1, c * 128:(c + 1) * 128],
                                lc[:, HC + 4:HC + 8], start=True, stop=True)
                        # logits l = es + ed_bc ; p = exp(lrelu(l) - ln64)
                        lg = pm.tile([128, cpb, 4], F32, tag="lg")
                        nc.vector.tensor_tensor(
                            lg[:], srcg[:, :, HC:HC + 4], ps_ed[:], OP.add)
                        lr = pm.tile([128, cpb, 4], F32, tag="lr")
                        nc.scalar.activation(lr[:], lg[:], AF.Prelu, alpha=SLOPE)
                        pf = pm.tile([128, cpb, 4], F32, tag="pf")
                        nc.scalar.activation(pf[:], lr[:], AF.Exp, bias=lnb_t[:])
                        p16 = pm.tile([128, cpb, 4], F16, tag="p16")
                        nc.scalar.activation(p16[:], pf[:], AF.Copy)

                        if debug and b == 0 and d == 0:
                            nc.sync.dma_start(dbg_p[:], p16[:].rearrange("p a b -> p (a b)"))
                            ped_sb = pm.tile([128, cpb, 4], F32, tag="pedsb")
                            nc.vector.tensor_copy(ped_sb[:], ps_ed[:])
                            nc.sync.dma_start(dbg_ped[:], ped_sb[:].rearrange("p a b -> p (a b)"))
                        # rows *= p (per-head broadcast multiply)
                        for c in range(cpb):
                            v = srcg[:, c, 0:HC].rearrange("p (h w) -> p h w", w=C)
                            nc.vector.tensor_tensor(
                                v, v,
                                p16[:, c, :].unsqueeze(2).to_broadcast((128, H, C)),
                                OP.mult)

                        if debug and b == 0 and d == 0:
                            nc.sync.dma_start(dbg_srcg2[:], srcg[:].rearrange("p a b -> p (a b)"))
                        # num/den accumulation
                        pnd = psn.tile([128, HC], F32, tag="pnd")
                        pden = pse.tile([128, 4], F32, tag="pden")
                        for c in range(cpb):
                            mc = mk[:, 0, c * 128:(c + 1) * 128]
                            nc.tensor.matmul(pnd[:, 0:HC], mc, srcg[:, c, 0:HC],
                                             start=(c == 0), stop=(c == cpb - 1))
                            nc.tensor.matmul(pden[:], mc, p16[:, c, :],
                                             start=(c == 0), stop=(c == cpb - 1))

                        if debug and b == 0 and d == 0:
                            pnd_sb = pm.tile([128, HC + 4], F32, tag="pndsb")
                            nc.vector.tensor_copy(pnd_sb[:, 0:HC], pnd[:])
                            nc.vector.tensor_copy(pnd_sb[:, HC:], pden[:])
                            nc.sync.dma_start(dbg_pnd[:], pnd_sb[:])
                        # self-loop p
                        sl = pm.tile([128, 4], F32, tag="sl")
                        nc.vector.tensor_tensor(
                            sl[:], lc[:, HC:HC + 4], lc[:, HC + 4:HC + 8], OP.add)
                        slr = pm.tile([128, 4], F32, tag="slr")
                        nc.scalar.activation(slr[:], sl[:], AF.Prelu, alpha=SLOPE)
                        psf = pm.tile([128, 4], F32, tag="psf")
                        nc.scalar.activation(psf[:], slr[:], AF.Exp, bias=lnb_t[:])

                        # normalize: stage = (num + p_self*h_loc) / (2*(den+p_self))
                        dtot = pm.tile([128, 4], F32, tag="dtot")
                        nc.vector.tensor_tensor(dtot[:], pden[:], psf[:],
                                                OP.add)
                        nc.vector.tensor_scalar(
                            out=dtot[:], in0=dtot[:], scalar1=2.0, scalar2=1e-30,
                            op0=OP.mult, op1=OP.max)
                        rec = pm.tile([128, 4], F32, tag="rec")
                        nc.vector.reciprocal(rec[:], dtot[:])

                        stg = po.tile([128, H, C], F32, tag=f"stg{d}", name=f"stg{d}")
                        for h in range(H):
                            nc.scalar.activation(
                                stg[:, h, :], lc[:, h * C:(h + 1) * C], AF.Copy,
                                scale=psf[:, h:h + 1])
                        nc.vector.tensor_tensor(
                            stg[:], stg[:],
                            pnd[:, 0:HC].rearrange("p (h w) -> p h w", w=C), OP.add)
                        for h in range(H):
                            nc.scalar.activation(
                                stg[:, h, :], stg[:, h, :], AF.Copy,
                                scale=rec[:, h:h + 1])
                        stage[d] = stg

                    ot = po.tile([128, HC], F32, tag="ot")
                    nc.vector.tensor_tensor(
                        ot[:].rearrange("p (h w) -> p h w", w=C),
                        stage[0][:], stage[1][:], OP.add)
                    nc.vector.tensor_tensor(ot[:], ot[:], bias_bc[:], OP.add)
                    nc.sync.dma_start(out[b * 128:(b + 1) * 128, :], ot[:])

    nc.compile()
    return nc


# ---------------------------------------------------------------- host side

def _wrap16(arr):
    """int idx array [n] -> dma_gather layout [128, n/16] int16 (replicated)."""
    n = len(arr)
    m = arr.reshape(n // 16, 16).astype(np.int16).T  # [16, n/16]
    return np.tile(m, (8, 1))


def prep_inputs(x, edge_index, W1, a_src1, a_dst1, b1, W2, a_src2, a_dst2, b2,
                nbin=NBIN, cb=CB):
    cpb = 2 * cb
    x = np.asarray(x, np.float32)
    ei = np.asarray(edge_index)
    src, dst = ei[0].astype(np.int64), ei[1].astype(np.int64)
    dirs = [(src, dst), (dst, src)]   # no self-loops; handled via local path

    # per-node degree by (dir, src-bank)
    deg = np.zeros((N, 4), np.int64)
    for j, (ss, dd) in enumerate(dirs):
        for bk in range(2):
            m = (ss >= B0REAL) == (bk == 1)
            deg[:, 2 * j + bk] = np.bincount(dd[m], minlength=N)

    # fp16 feature layouts
    x16 = x.astype(np.float16)
    xpad = np.zeros((NT * 128, DIN), np.float16)
    xpad[0:B0REAL] = x16[0:B0REAL]
    xpad[BKROWS:BKROWS + (N - B0REAL)] = x16[B0REAL:N]
    xTb = np.ascontiguousarray(xpad.T.reshape(2, 128, NT * 128))

    W_l = [np.asarray(W1, np.float32), np.asarray(W2, np.float32)]
    a_l = [(np.asarray(a_src1, np.float32), np.asarray(a_dst1, np.float32)),
           (np.asarray(a_src2, np.float32), np.asarray(a_dst2, np.float32))]
    Wsb = np.zeros((2, 2, 128, HC), np.float16)
    for d in range(2):
        for k in range(2):
            Wsb[d, k] = W_l[d][k * 128:(k + 1) * 128, :].astype(np.float16)
    cols = []
    for d in range(2):
        for a in a_l[d]:
            A = np.zeros((HC, H), np.float32)
            for h in range(H):
                A[h * C:(h + 1) * C, h] = a[h]
            cols.append(W_l[d] @ A)         # [256, 4]
    wsd_full = np.concatenate(cols, axis=1)  # [256, 16] (es1|ed1|es2|ed2)
    wsd_in = np.ascontiguousarray(
        wsd_full.reshape(2, 128, 16)).astype(np.float16)
    b_in = (0.5 * (np.asarray(b1) + np.asarray(b2))).astype(np.float32)
    b_in = b_in.reshape(1, HC)

    in_maps, perms = [], []
    for core in range(NCORES):
        lo = core * NPC
        nodes = np.arange(lo, lo + NPC)
        order = nodes[np.argsort(-deg[nodes].sum(1), kind="stable")]
        degs = deg[order]
        bins_load = np.zeros((nbin, 4), np.int64)
        bins_cnt = np.zeros(nbin, np.int64)
        node_blk = np.full(N, -1, np.int64)
        node_slot = np.full(N, -1, np.int64)
        for i_n in range(len(order)):
            dgl = degs[i_n]
            ok = (bins_cnt < 128) & ((bins_load + dgl) <= cb * 128).all(1)
            assert ok.any(), "bin packing failed; raise nbin/cb"
            cand = np.where(ok)[0]
            nl = (bins_load[cand] + dgl).max(1) * 1000 + bins_cnt[cand]
            i = cand[np.argmin(nl)]
            node_blk[order[i_n]] = i
            node_slot[order[i_n]] = bins_cnt[i]
            bins_load[i] += dgl
            bins_cnt[i] += 1

        perm = np.full(nbin * 128, -1, np.int64)
        perm[node_blk[nodes] * 128 + node_slot[nodes]] = nodes
        perms.append(perm)

        g_idx = np.zeros((2, nbin, 128, 16 * cb), np.int16)
        m_host = np.zeros((2, nbin, 2, 128, cpb * 128), np.float16)
        for d, (ss, dd) in enumerate(dirs):
            sel = (dd >= lo) & (dd < lo + NPC)
            es_, ed_ = ss[sel], dd[sel]
            blk = node_blk[ed_]
            bank = (es_ >= B0REAL).astype(np.int64)
            eo = np.lexsort((bank, blk))
            es_, ed_, blk, bank = es_[eo], ed_[eo], blk[eo], bank[eo]
            seg = blk * 2 + bank
            segbnd = np.flatnonzero(np.diff(seg, prepend=-1))
            within = np.arange(len(seg)) - np.repeat(segbnd, np.diff(
                np.append(segbnd, len(seg))))
            assert (within < cb * 128).all()
            slot = within + bank * (cb * 128)
            srcrel = np.where(bank == 0, es_, es_ - B0REAL)
            dslot = node_slot[ed_]
            s_idx = np.full((nbin, cpb * 128), PADIDX, np.int64)
            s_idx[blk, slot] = srcrel
            chunk, epart = slot // 128, slot % 128
            m_host[d, blk, 0, epart, chunk * 128 + dslot] = 1.0
            m_host[d, blk, 1, dslot, chunk * 128 + epart] = 1.0
            for bb in range(nbin):
                g_idx[d, bb, :, 0:8 * cb] = _wrap16(s_idx[bb, 0:cb * 128])
                g_idx[d, bb, :, 8 * cb:16 * cb] = _wrap16(s_idx[bb, cb * 128:])

        xloc = np.zeros((nbin * 128, DIN), np.float16)
        valid = perm >= 0
        xloc[valid] = x16[perm[valid]]
        xTl = np.ascontiguousarray(xloc.T.reshape(2, 128, nbin * 128))

        in_maps.append({
            "xTb": xTb, "xTl": xTl, "Wsb": Wsb, "wsd_in": wsd_in, "b_in": b_in,
            "gidx": g_idx, "msk": m_host,
        })
    return in_maps, perms


_NC_CACHE = {}


def kernel(**inputs):
    in_maps, perms = prep_inputs(**inputs)
    key = (NBIN, CB)
    if key not in _NC_CACHE:
        _NC_CACHE[key] = build_kernel(NBIN, CB)
    nc = _NC_CACHE[key]
    res = run_bass_kernel_spmd(nc, in_maps, list(range(NCORES)))
    result = np.empty((N, HC), np.float32)
    for core in range(NCORES):
        o = res.results[core]["out"]
        p = perms[core]
        valid = p >= 0
        result[p[valid]] = o[valid]
    return result


# revision 15
# speedup vs baseline: 2.5389x; 1.1687x over previous
"""DirGATConv on 8 Trainium2 NeuronCores (Bass/Tile), v2.

Strategy (node/data parallel, no collectives):
  - Each core owns 6250 destination nodes, permuted into NBIN blocks of <=128
    by bin packing so every (block, direction, src-bank) has at most CB*128
    non-self-loop edges.
  - Phase A (replicated on every core): h = x @ W_d for all nodes plus the
    per-node attention projections es/ed = x @ (W_d a_*), written to two DRAM
    gather tables per direction (fp16 rows: 256 h | 4 es | 124 pad = 768 B;
    row count per bank <= 32767 because dma_gather indices are int16).  A
    bin-permuted local table per direction holds (h | es | ed) for the core's
    own destinations (544 B rows, read linearly in Phase B).
  - Phase B per (block, direction): dma_gather the source rows (one gather
    per src-bank), then with host-shipped 0/1 fp16 masks M [e,d] / MT [d,e]:
      ed_bc  = MT^T @ ed_tile                    (per-edge dst projection)
      p      = exp(lrelu(es + ed_bc) - ln 64)    (scalar engine; -ln64 keeps
                                                  h*p inside fp16 range)
      rows  *= p (per-head broadcast multiply), then one matmul per chunk
      num    = M^T @ rows, den = M^T @ p         (same stationary mask)
      out_d  = (num + p_self*h_loc) / (2*(den + p_self))
    Softmax normalization is exact because num and den are linear in p and
    any per-edge common factor (the -ln64 bias) cancels in num/den.
  - Host work is graph-structure-only (bin packing, gather indices, masks,
    layout transposes) plus standard weight fusion (W @ a projections).
"""

import numpy as np

import concourse.bacc as bacc
import concourse.mybir as mybir
import concourse.tile as tile
from concourse.bass_utils import run_bass_kernel_spmd
from concourse import library_config

# problem constants
N, E, DIN, H, C = 50000, 400000, 256, 4, 64
HC = H * C
ALPHA, SLOPE = 0.5, 0.2

# distribution constants
NCORES = 8
NPC = N // NCORES              # 6250 destinations per core
B0REAL = 24960                 # real nodes in bank 0 (nodes 0..24959)
BKROWS = 25088                 # rows per table bank (includes zero pad rows)
PADIDX = BKROWS - 1            # gather index for empty edge slots (zero row)
NT = 392                       # main node tiles (2 banks x 196)
NBIN = 51                      # destination blocks per core
CB = 5                         # gather chunks per (block, src-bank)
CPB = 2 * CB                   # chunks per block
NLOC = NBIN * 128
TW = 384                       # table row width (fp16) = 768 B
LW = 272                       # local row width (fp16) = 544 B
LNB = float(np.log(64.0))      # exp bias, cancels in num/den
F16 = mybir.dt.float16
F32 = mybir.dt.float32
I16 = mybir.dt.int16
AF = mybir.ActivationFunctionType
OP = mybir.AluOpType


def build_kernel(nbin=NBIN, cb=CB, debug=False):
    cpb = 2 * cb
    nc = bacc.Bacc("TRN2", num_swdge_queues=4)
    if debug:
        dbg_srcg = nc.dram_tensor("dbg_srcg", [128, cpb * TW], F16, kind="ExternalOutput")
        dbg_ped = nc.dram_tensor("dbg_ped", [128, cpb * 4], F32, kind="ExternalOutput")
        dbg_p = nc.dram_tensor("dbg_p", [128, cpb * 4], F16, kind="ExternalOutput")
        dbg_pnd = nc.dram_tensor("dbg_pnd", [128, HC + 4], F32, kind="ExternalOutput")
        dbg_lc = nc.dram_tensor("dbg_lc", [128, LW], F16, kind="ExternalOutput")
        dbg_srcg2 = nc.dram_tensor("dbg_srcg2", [128, cpb * TW], F16, kind="ExternalOutput")

    xTb = nc.dram_tensor("xTb", [2, 128, NT * 128], F16, kind="ExternalInput")
    xTl = nc.dram_tensor("xTl", [2, 128, nbin * 128], F16, kind="ExternalInput")
    Wsb = nc.dram_tensor("Wsb", [2, 2, 128, HC + 8], F16, kind="ExternalInput")
    b_in = nc.dram_tensor("b_in", [1, HC], F32, kind="ExternalInput")
    gidx = nc.dram_tensor("gidx", [2, nbin, 128, 16 * cb], I16, kind="ExternalInput")
    msk = nc.dram_tensor("msk", [2, nbin, 2, 128, cpb * 128], F16, kind="ExternalInput")
    out = nc.dram_tensor("out", [nbin * 128, HC], F32, kind="ExternalOutput")

    with tile.TileContext(nc) as tc:
        with (
            tc.tile_pool(name="dram", bufs=1, space="DRAM") as dpool,
            tc.tile_pool(name="const", bufs=1) as cpool,
        ):
            nc.gpsimd.load_library(library_config.mlp)

            tabs = [
                [dpool.tile([BKROWS, TW], F16, tag=f"tab{d}{k}", name=f"tab{d}{k}")
                 for k in range(2)]
                for d in range(2)
            ]
            locs = [dpool.tile([nbin * 128, LW], F16, tag=f"loc{d}", name=f"loc{d}")
                    for d in range(2)]

            # weights: w_sb[d] [128 din, 2 k, 264 = hc|es|ed]
            w_sb = [cpool.tile([128, 2, HC + 8], F16, tag=f"w{d}", name=f"w{d}")
                    for d in range(2)]
            for d in range(2):
                nc.sync.dma_start(
                    w_sb[d][:], Wsb[d].rearrange("k p c -> p k c"))
            bias_bc = cpool.tile([128, HC], F32)
            nc.sync.dma_start(bias_bc[:], b_in[:].to_broadcast((128, HC)))
            lnb_t = cpool.tile([128, 1], F32)
            nc.vector.memset(lnb_t[:], -LNB)

            # ---------------- Phase A ----------------
            with (
                tc.tile_pool(name="pAx", bufs=2) as pax,
                tc.tile_pool(name="pAs", bufs=2) as pas,
                tc.tile_pool(name="psA", bufs=2, space="PSUM") as psa,
            ):
                st = [None, None]

                def node_tile(xt_k, j, wide):
                    """One 128-node tile: xt_k [128, 2, 128]; write into
                    st[d][:, j, :] (wide=TW) or st[d] [128, LW] (wide=LW)."""
                    ph0 = psa.tile([128, HC + 8], F32, tag="ph0")
                    ph1 = psa.tile([128, HC + 8], F32, tag="ph1")
                    ph = [ph0, ph1]
                    for k in range(2):
                        for d in range(2):
                            nc.tensor.matmul(
                                ph[d][:], xt_k[:, k, :], w_sb[d][:, k, :],
                                start=(k == 0), stop=(k == 1))
                    if wide == TW:
                        nc.vector.tensor_copy(st[0][:, j, 0:HC + 8], ph[0][:])
                        nc.scalar.activation(st[1][:, j, 0:HC + 8], ph[1][:], AF.Copy)
                    else:
                        nc.vector.tensor_copy(st[0][:, 0:HC + 8], ph[0][:])
                        nc.scalar.activation(st[1][:, 0:HC + 8], ph[1][:], AF.Copy)

                # main tiles: 4-tile batches (bank boundary at tile 196 = 49*4)
                for it in range(NT // 4):
                    xt = pax.tile([128, 4, 2, 128], F16, tag="xt")
                    for k in range(2):
                        nc.sync.dma_start(
                            xt[:, :, k, :],
                            xTb[k, :, it * 512:(it + 1) * 512].rearrange(
                                "p (t c) -> p t c", c=128))
                    for d in range(2):
                        st[d] = pas.tile([128, 4, TW], F16, tag=f"st{d}", name=f"st{d}")
                    for t in range(4):
                        node_tile(xt[:, t, :, :], t, TW)
                    t0 = it * 4
                    bk = 0 if t0 < 196 else 1
                    r0 = (t0 - (0 if bk == 0 else 196)) * 128
                    for d in range(2):
                        dst = tabs[d][bk][r0:r0 + 512, :].rearrange(
                            "(t p) c -> p t c", t=4)
                        eng = nc.gpsimd if d == 0 else nc.scalar
                        eng.dma_start(dst, st[d][:])

                # local tiles (one per iteration)
                for t in range(nbin):
                    xt = pax.tile([128, 1, 2, 128], F16, tag="xt")
                    for k in range(2):
                        nc.sync.dma_start(
                            xt[:, 0, k, :],
                            xTl[k, :, t * 128:(t + 1) * 128])
                    for d in range(2):
                        st[d] = pas.tile([128, LW], F16, tag=f"lst{d}", name=f"lst{d}")
                    node_tile(xt[:, 0, :, :], 0, LW)
                    for d in range(2):
                        eng = nc.gpsimd if d == 0 else nc.scalar
                        eng.dma_start(locs[d][t * 128:(t + 1) * 128, :], st[d][:])

            # ---------------- Phase B ----------------
            with (
                tc.tile_pool(name="pBg", bufs=5) as pg,
                tc.tile_pool(name="pBk", bufs=4) as pk,
                tc.tile_pool(name="pBm", bufs=6) as pm,
                tc.tile_pool(name="pBo", bufs=2) as po,
                tc.tile_pool(name="psN", bufs=3, space="PSUM") as psn,
                tc.tile_pool(name="psE", bufs=2, space="PSUM") as pse,
            ):
                for b in range(nbin):
                    stage = [None, None]
                    for d in range(2):
                        gi = pm.tile([128, 16 * cb], I16, tag="gi")
                        nc.sync.dma_start(gi[:], gidx[d, b])
                        mk = pk.tile([128, 2, cpb * 128], F16, tag="mk")
                        nc.sync.dma_start(mk[:], msk[d, b].rearrange("m p c -> p m c"))
                        lc = pm.tile([128, LW], F16, tag="lc")
                        nc.sync.dma_start(lc[:], locs[d][b * 128:(b + 1) * 128, :])

                        srcg = pg.tile([128, cpb, TW], F16, tag="srcg")
                        for half in range(2):
                            nc.gpsimd.dma_gather(
                                srcg[:, half * cb:(half + 1) * cb, :],
                                tabs[d][half][:],
                                gi[:, half * 8 * cb:(half + 1) * 8 * cb],
                                cb * 128, cb * 128, TW,
                                queue_num=(2 * (2 * b + d) + half) % 4,
                                single_packet=False)

                        if debug and b == 0 and d == 0:
                            nc.sync.dma_start(dbg_srcg[:], srcg[:].rearrange("p a b -> p (a b)"))
                            nc.sync.dma_start(dbg_lc[:], lc[:])
                        # ed_bc[e, h] via MT^T @ ed_tile
                        ps_ed = pse.tile([128, cpb, 4], F32, tag="ped")
                        for c in range(cpb):
                            nc.tensor.matmul(
                                ps_ed[:, c, :], mk[:, 1, c * 128:(c + 1) * 128],
                                lc[:, HC + 4:HC + 8], start=True, stop=True)
                        # logits l = es + ed_bc ; p = exp(lrelu(l) - ln64)
                        lg = pm.tile([128, cpb, 4], F32, tag="lg")
                        nc.vector.tensor_tensor(
                            lg[:], srcg[:, :, HC:HC + 4], ps_ed[:], OP.add)
                        lr = pm.tile([128, cpb, 4], F32, tag="lr")
                        nc.scalar.activation(lr[:], lg[:], AF.Prelu, alpha=SLOPE)
                        pf = pm.tile([128, cpb, 4], F32, tag="pf")
                        nc.scalar.activation(pf[:], lr[:], AF.Exp, bias=lnb_t[:])
                        p16 = srcg[:, :, HC + 4:HC + 8]
                        nc.scalar.activation(p16, pf[:], AF.Copy)

                        if debug and b == 0 and d == 0:
                            nc.sync.dma_start(dbg_p[:], p16[:].rearrange("p a b -> p (a b)"))
                            ped_sb = pm.tile([128, cpb, 4], F32, tag="pedsb")
                            nc.vector.tensor_copy(ped_sb[:], ps_ed[:])
                            nc.sync.dma_start(dbg_ped[:], ped_sb[:].rearrange("p a b -> p (a b)"))
                        # rows *= p (per-head broadcast multiply)
                        for c in range(cpb):
                            v = srcg[:, c, 0:HC].rearrange("p (h w) -> p h w", w=C)
                            nc.vector.tensor_tensor(
                                v, v,
                                srcg[:, c, HC + 4:HC + 8].unsqueeze(2)
                                .to_broadcast((128, H, C)),
                                OP.mult)

                        if debug and b == 0 and d == 0:
                            nc.sync.dma_start(dbg_srcg2[:], srcg[:].rearrange("p a b -> p (a b)"))
                        # num/den accumulation
                        pnd = psn.tile([128, HC + 8], F32, tag="pnd")
                        for c in range(cpb):
                            mc = mk[:, 0, c * 128:(c + 1) * 128]
                            nc.tensor.matmul(pnd[:, 0:HC + 8], mc,
                                             srcg[:, c, 0:HC + 8],
                                             start=(c == 0), stop=(c == cpb - 1))

                        if debug and b == 0 and d == 0:
                            pnd_sb = pm.tile([128, HC + 4], F32, tag="pndsb")
                            nc.vector.tensor_copy(pnd_sb[:, 0:HC], pnd[:, 0:HC])
                            nc.vector.tensor_copy(pnd_sb[:, HC:], pnd[:, HC + 4:HC + 8])
                            nc.sync.dma_start(dbg_pnd[:], pnd_sb[:])
                        # self-loop p
                        sl = pm.tile([128, 4], F32, tag="sl")
                        nc.vector.tensor_tensor(
                            sl[:], lc[:, HC:HC + 4], lc[:, HC + 4:HC + 8], OP.add)
                        slr = pm.tile([128, 4], F32, tag="slr")
                        nc.scalar.activation(slr[:], sl[:], AF.Prelu, alpha=SLOPE)
                        psf = pm.tile([128, 4], F32, tag="psf")
                        nc.scalar.activation(psf[:], slr[:], AF.Exp, bias=lnb_t[:])

                        # normalize: stage = (num + p_self*h_loc) / (2*(den+p_self))
                        dtot = pm.tile([128, 4], F32, tag="dtot")
                        nc.vector.tensor_tensor(dtot[:], pnd[:, HC + 4:HC + 8], psf[:],
                                                OP.add)
                        nc.vector.tensor_scalar(
                            out=dtot[:], in0=dtot[:], scalar1=2.0, scalar2=1e-30,
                            op0=OP.mult, op1=OP.max)
                        rec = pm.tile([128, 4], F32, tag="rec")
                        nc.vector.reciprocal(rec[:], dtot[:])

                        stg = po.tile([128, H, C], F32, tag=f"stg{d}", name=f"stg{d}")
                        for h in range(H):
                            nc.scalar.activation(
                                stg[:, h, :], lc[:, h * C:(h + 1) * C], AF.Copy,
                                scale=psf[:, h:h + 1])
                        nc.vector.tensor_tensor(
                            stg[:], stg[:],
                            pnd[:, 0:HC].rearrange("p (h w) -> p h w", w=C), OP.add)
                        for h in range(H):
                            nc.scalar.activation(
                                stg[:, h, :], stg[:, h, :], AF.Copy,
                                scale=rec[:, h:h + 1])
                        stage[d] = stg

                    ot = po.tile([128, HC], F32, tag="ot")
                    nc.vector.tensor_tensor(
                        ot[:].rearrange("p (h w) -> p h w", w=C),
                        stage[0][:], stage[1][:], OP.add)
                    nc.vector.tensor_tensor(ot[:], ot[:], bias_bc[:], OP.add)
                    nc.sync.dma_start(out[b * 128:(b + 1) * 128, :], ot[:])

    nc.compile()
    return nc


# ---------------------------------------------------------------- host side

def _wrap16(arr):
    """int idx array [n] -> dma_gather layout [128, n/16] int16 (replicated)."""
    n = len(arr)
    m = arr.reshape(n // 16, 16).astype(np.int16).T  # [16, n/16]
    return np.tile(m, (8, 1))


def prep_inputs(x, edge_index, W1, a_src1, a_dst1, b1, W2, a_src2, a_dst2, b2,
                nbin=NBIN, cb=CB):
    cpb = 2 * cb
    x = np.asarray(x, np.float32)
    ei = np.asarray(edge_index)
    src, dst = ei[0].astype(np.int64), ei[1].astype(np.int64)
    dirs = [(src, dst), (dst, src)]   # no self-loops; handled via local path

    # per-node degree by (dir, src-bank)
    deg = np.zeros((N, 4), np.int64)
    for j, (ss, dd) in enumerate(dirs):
        for bk in range(2):
            m = (ss >= B0REAL) == (bk == 1)
            deg[:, 2 * j + bk] = np.bincount(dd[m], minlength=N)

    # fp16 feature layouts
    x16 = x.astype(np.float16)
    xpad = np.zeros((NT * 128, DIN), np.float16)
    xpad[0:B0REAL] = x16[0:B0REAL]
    xpad[BKROWS:BKROWS + (N - B0REAL)] = x16[B0REAL:N]
    xTb = np.ascontiguousarray(xpad.T.reshape(2, 128, NT * 128))

    W_l = [np.asarray(W1, np.float32), np.asarray(W2, np.float32)]
    a_l = [(np.asarray(a_src1, np.float32), np.asarray(a_dst1, np.float32)),
           (np.asarray(a_src2, np.float32), np.asarray(a_dst2, np.float32))]
    cols = []
    for d in range(2):
        for a in a_l[d]:
            A = np.zeros((HC, H), np.float32)
            for h in range(H):
                A[h * C:(h + 1) * C, h] = a[h]
            cols.append(W_l[d] @ A)         # [256, 4]
    Wsb = np.zeros((2, 2, 128, HC + 8), np.float16)
    for d in range(2):
        wext = np.concatenate([W_l[d], cols[2 * d], cols[2 * d + 1]],
                              axis=1).astype(np.float16)  # [256, 264]
        for k in range(2):
            Wsb[d, k] = wext[k * 128:(k + 1) * 128, :]
    b_in = (0.5 * (np.asarray(b1) + np.asarray(b2))).astype(np.float32)
    b_in = b_in.reshape(1, HC)

    in_maps, perms = [], []
    for core in range(NCORES):
        lo = core * NPC
        nodes = np.arange(lo, lo + NPC)
        order = nodes[np.argsort(-deg[nodes].sum(1), kind="stable")]
        degs = deg[order]
        bins_load = np.zeros((nbin, 4), np.int64)
        bins_cnt = np.zeros(nbin, np.int64)
        node_blk = np.full(N, -1, np.int64)
        node_slot = np.full(N, -1, np.int64)
        for i_n in range(len(order)):
            dgl = degs[i_n]
            ok = (bins_cnt < 128) & ((bins_load + dgl) <= cb * 128).all(1)
            assert ok.any(), "bin packing failed; raise nbin/cb"
            cand = np.where(ok)[0]
            nl = (bins_load[cand] + dgl).max(1) * 1000 + bins_cnt[cand]
            i = cand[np.argmin(nl)]
            node_blk[order[i_n]] = i
            node_slot[order[i_n]] = bins_cnt[i]
            bins_load[i] += dgl
            bins_cnt[i] += 1

        perm = np.full(nbin * 128, -1, np.int64)
        perm[node_blk[nodes] * 128 + node_slot[nodes]] = nodes
        perms.append(perm)

        g_idx = np.zeros((2, nbin, 128, 16 * cb), np.int16)
        m_host = np.zeros((2, nbin, 2, 128, cpb * 128), np.float16)
        for d, (ss, dd) in enumerate(dirs):
            sel = (dd >= lo) & (dd < lo + NPC)
            es_, ed_ = ss[sel], dd[sel]
            blk = node_blk[ed_]
            bank = (es_ >= B0REAL).astype(np.int64)
            eo = np.lexsort((bank, blk))
            es_, ed_, blk, bank = es_[eo], ed_[eo], blk[eo], bank[eo]
            seg = blk * 2 + bank
            segbnd = np.flatnonzero(np.diff(seg, prepend=-1))
            within = np.arange(len(seg)) - np.repeat(segbnd, np.diff(
                np.append(segbnd, len(seg))))
            assert (within < cb * 128).all()
            slot = within + bank * (cb * 128)
            srcrel = np.where(bank == 0, es_, es_ - B0REAL)
            dslot = node_slot[ed_]
            s_idx = np.full((nbin, cpb * 128), PADIDX, np.int64)
            s_idx[blk, slot] = srcrel
            chunk, epart = slot // 128, slot % 128
            m_host[d, blk, 0, epart, chunk * 128 + dslot] = 1.0
            m_host[d, blk, 1, dslot, chunk * 128 + epart] = 1.0
            for bb in range(nbin):
                g_idx[d, bb, :, 0:8 * cb] = _wrap16(s_idx[bb, 0:cb * 128])
                g_idx[d, bb, :, 8 * cb:16 * cb] = _wrap16(s_idx[bb, cb * 128:])

        xloc = np.zeros((nbin * 128, DIN), np.float16)
        valid = perm >= 0
        xloc[valid] = x16[perm[valid]]
        xTl = np.ascontiguousarray(xloc.T.reshape(2, 128, nbin * 128))

        in_maps.append({
            "xTb": xTb, "xTl": xTl, "Wsb": Wsb, "b_in": b_in,
            "gidx": g_idx, "msk": m_host,
        })
    return in_maps, perms


_NC_CACHE = {}


def kernel(**inputs):
    in_maps, perms = prep_inputs(**inputs)
    key = (NBIN, CB)
    if key not in _NC_CACHE:
        _NC_CACHE[key] = build_kernel(NBIN, CB)
    nc = _NC_CACHE[key]
    res = run_bass_kernel_spmd(nc, in_maps, list(range(NCORES)))
    result = np.empty((N, HC), np.float32)
    for core in range(NCORES):
        o = res.results[core]["out"]
        p = perms[core]
        valid = p >= 0
        result[p[valid]] = o[valid]
    return result


# revision 16
# speedup vs baseline: 3.2333x; 1.2735x over previous
"""DirGATConv on 8 Trainium2 NeuronCores (Bass/Tile), v2.

Strategy (node/data parallel, no collectives):
  - Each core owns 6250 destination nodes, permuted into NBIN blocks of <=128
    by bin packing so every (block, direction, src-bank) has at most CB*128
    non-self-loop edges.
  - Phase A (replicated on every core): h = x @ W_d for all nodes plus the
    per-node attention projections es/ed = x @ (W_d a_*), written to two DRAM
    gather tables per direction (fp16 rows: 256 h | 4 es | 124 pad = 768 B;
    row count per bank <= 32767 because dma_gather indices are int16).  A
    bin-permuted local table per direction holds (h | es | ed) for the core's
    own destinations (544 B rows, read linearly in Phase B).
  - Phase B per (block, direction): dma_gather the source rows (one gather
    per src-bank), then with host-shipped 0/1 fp16 masks M [e,d] / MT [d,e]:
      ed_bc  = MT^T @ ed_tile                    (per-edge dst projection)
      p      = exp(lrelu(es + ed_bc) - ln 64)    (scalar engine; -ln64 keeps
                                                  h*p inside fp16 range)
      rows  *= p (per-head broadcast multiply), then one matmul per chunk
      num    = M^T @ rows, den = M^T @ p         (same stationary mask)
      out_d  = (num + p_self*h_loc) / (2*(den + p_self))
    Softmax normalization is exact because num and den are linear in p and
    any per-edge common factor (the -ln64 bias) cancels in num/den.
  - Host work is graph-structure-only (bin packing, gather indices, masks,
    layout transposes) plus standard weight fusion (W @ a projections).
"""

import numpy as np

import concourse.bacc as bacc
import concourse.mybir as mybir
import concourse.tile as tile
from concourse.bass_utils import run_bass_kernel_spmd
from concourse import library_config

# problem constants
N, E, DIN, H, C = 50000, 400000, 256, 4, 64
HC = H * C
ALPHA, SLOPE = 0.5, 0.2

# distribution constants
NCORES = 8
NPC = N // NCORES              # 6250 destinations per core
B0REAL = 24960                 # real nodes in bank 0 (nodes 0..24959)
BKROWS = 25088                 # rows per table bank (includes zero pad rows)
PADIDX = BKROWS - 1            # gather index for empty edge slots (zero row)
NT = 392                       # main node tiles (2 banks x 196)
NBIN = 50                      # destination blocks per core
CB = 4                         # gather chunks per (block, src-bank)
CPB = 2 * CB                   # chunks per block
NLOC = NBIN * 128
TW = 384                       # table row width (fp16) = 768 B
LW = 272                       # local row width (fp16) = 544 B
LNB = float(np.log(64.0))      # exp bias, cancels in num/den
F16 = mybir.dt.float16
F32 = mybir.dt.float32
I16 = mybir.dt.int16
AF = mybir.ActivationFunctionType
OP = mybir.AluOpType


def build_kernel(nbin=NBIN, cb=CB, debug=False):
    cpb = 2 * cb
    nc = bacc.Bacc("TRN2", num_swdge_queues=4)
    if debug:
        dbg_srcg = nc.dram_tensor("dbg_srcg", [128, cpb * TW], F16, kind="ExternalOutput")
        dbg_ped = nc.dram_tensor("dbg_ped", [128, cpb * 4], F32, kind="ExternalOutput")
        dbg_p = nc.dram_tensor("dbg_p", [128, cpb * 4], F16, kind="ExternalOutput")
        dbg_pnd = nc.dram_tensor("dbg_pnd", [128, HC + 4], F32, kind="ExternalOutput")
        dbg_lc = nc.dram_tensor("dbg_lc", [128, LW], F16, kind="ExternalOutput")
        dbg_srcg2 = nc.dram_tensor("dbg_srcg2", [128, cpb * TW], F16, kind="ExternalOutput")

    xTb = nc.dram_tensor("xTb", [2, 128, NT * 128], F16, kind="ExternalInput")
    xTl = nc.dram_tensor("xTl", [2, 128, nbin * 128], F16, kind="ExternalInput")
    Wsb = nc.dram_tensor("Wsb", [2, 2, 128, HC + 8], F16, kind="ExternalInput")
    b_in = nc.dram_tensor("b_in", [1, HC], F32, kind="ExternalInput")
    gidx = nc.dram_tensor("gidx", [2, nbin, 128, 16 * cb], I16, kind="ExternalInput")
    msk = nc.dram_tensor("msk", [2, nbin, 2, 128, cpb * 128], F16, kind="ExternalInput")
    out = nc.dram_tensor("out", [nbin * 128, HC], F32, kind="ExternalOutput")

    with tile.TileContext(nc) as tc:
        with (
            tc.tile_pool(name="dram", bufs=1, space="DRAM") as dpool,
            tc.tile_pool(name="const", bufs=1) as cpool,
        ):
            nc.gpsimd.load_library(library_config.mlp)

            tabs = [
                [dpool.tile([BKROWS, TW], F16, tag=f"tab{d}{k}", name=f"tab{d}{k}")
                 for k in range(2)]
                for d in range(2)
            ]
            locs = [dpool.tile([nbin * 128, LW], F16, tag=f"loc{d}", name=f"loc{d}")
                    for d in range(2)]

            # weights: w_sb[d] [128 din, 2 k, 264 = hc|es|ed]
            w_sb = [cpool.tile([128, 2, HC + 8], F16, tag=f"w{d}", name=f"w{d}")
                    for d in range(2)]
            for d in range(2):
                nc.sync.dma_start(
                    w_sb[d][:], Wsb[d].rearrange("k p c -> p k c"))
            bias_bc = cpool.tile([128, HC], F32)
            nc.sync.dma_start(bias_bc[:], b_in[:].to_broadcast((128, HC)))
            lnb_t = cpool.tile([128, 1], F32)
            nc.vector.memset(lnb_t[:], -LNB)

            # ---------------- Phase A ----------------
            with (
                tc.tile_pool(name="pAx", bufs=3) as pax,
                tc.tile_pool(name="pAs", bufs=3) as pas,
                tc.tile_pool(name="psA", bufs=3, space="PSUM") as psa,
            ):
                st = [None, None]

                def node_tile(xt_k, j, wide):
                    """One 128-node tile: xt_k [128, 2, 128]; write into
                    st[d][:, j, :] (wide=TW) or st[d] [128, LW] (wide=LW)."""
                    ph0 = psa.tile([128, HC + 8], F32, tag="ph0")
                    ph1 = psa.tile([128, HC + 8], F32, tag="ph1")
                    ph = [ph0, ph1]
                    for k in range(2):
                        for d in range(2):
                            nc.tensor.matmul(
                                ph[d][:], xt_k[:, k, :], w_sb[d][:, k, :],
                                start=(k == 0), stop=(k == 1))
                    if wide == TW:
                        nc.vector.tensor_copy(st[0][:, j, 0:HC + 8], ph[0][:])
                        nc.scalar.activation(st[1][:, j, 0:HC + 8], ph[1][:], AF.Copy)
                    else:
                        nc.vector.tensor_copy(st[0][:, 0:HC + 8], ph[0][:])
                        nc.scalar.activation(st[1][:, 0:HC + 8], ph[1][:], AF.Copy)

                # main tiles: 4-tile batches (bank boundary at tile 196 = 49*4)
                for it in range(NT // 4):
                    xt = pax.tile([128, 4, 2, 128], F16, tag="xt")
                    for k in range(2):
                        nc.sync.dma_start(
                            xt[:, :, k, :],
                            xTb[k, :, it * 512:(it + 1) * 512].rearrange(
                                "p (t c) -> p t c", c=128))
                    for d in range(2):
                        st[d] = pas.tile([128, 4, TW], F16, tag=f"st{d}", name=f"st{d}")
                    for t in range(4):
                        node_tile(xt[:, t, :, :], t, TW)
                    t0 = it * 4
                    bk = 0 if t0 < 196 else 1
                    r0 = (t0 - (0 if bk == 0 else 196)) * 128
                    for d in range(2):
                        dst = tabs[d][bk][r0:r0 + 512, :].rearrange(
                            "(t p) c -> p t c", t=4)
                        eng = nc.gpsimd if d == 0 else nc.scalar
                        eng.dma_start(dst, st[d][:])

                # local tiles (one per iteration)
                for t in range(nbin):
                    xt = pax.tile([128, 1, 2, 128], F16, tag="xt")
                    for k in range(2):
                        nc.sync.dma_start(
                            xt[:, 0, k, :],
                            xTl[k, :, t * 128:(t + 1) * 128])
                    for d in range(2):
                        st[d] = pas.tile([128, LW], F16, tag=f"lst{d}", name=f"lst{d}")
                    node_tile(xt[:, 0, :, :], 0, LW)
                    for d in range(2):
                        eng = nc.gpsimd if d == 0 else nc.scalar
                        eng.dma_start(locs[d][t * 128:(t + 1) * 128, :], st[d][:])

            # ---------------- Phase B ----------------
            with (
                tc.tile_pool(name="pBg", bufs=5) as pg,
                tc.tile_pool(name="pBk", bufs=4) as pk,
                tc.tile_pool(name="pBm", bufs=6) as pm,
                tc.tile_pool(name="pBo", bufs=2) as po,
                tc.tile_pool(name="psN", bufs=3, space="PSUM") as psn,
                tc.tile_pool(name="psE", bufs=2, space="PSUM") as pse,
            ):
                for b in range(nbin):
                    stage = [None, None]
                    for d in range(2):
                        gi = pm.tile([128, 16 * cb], I16, tag="gi")
                        nc.sync.dma_start(gi[:], gidx[d, b])
                        mk = pk.tile([128, 2, cpb * 128], F16, tag="mk")
                        nc.scalar.dma_start(mk[:], msk[d, b].rearrange("m p c -> p m c"))
                        lc = pm.tile([128, LW], F16, tag="lc")
                        nc.sync.dma_start(lc[:], locs[d][b * 128:(b + 1) * 128, :])

                        srcg = pg.tile([128, cpb, TW], F16, tag="srcg")
                        for half in range(2):
                            nc.gpsimd.dma_gather(
                                srcg[:, half * cb:(half + 1) * cb, :],
                                tabs[d][half][:],
                                gi[:, half * 8 * cb:(half + 1) * 8 * cb],
                                cb * 128, cb * 128, TW,
                                queue_num=(2 * (2 * b + d) + half) % 4,
                                single_packet=False)

                        if debug and b == 0 and d == 0:
                            nc.sync.dma_start(dbg_srcg[:], srcg[:].rearrange("p a b -> p (a b)"))
                            nc.sync.dma_start(dbg_lc[:], lc[:])
                        # ed_bc[e, h] via MT^T @ ed_tile
                        ps_ed = pse.tile([128, cpb, 4], F32, tag="ped")
                        for c in range(cpb):
                            nc.tensor.matmul(
                                ps_ed[:, c, :], mk[:, 1, c * 128:(c + 1) * 128],
                                lc[:, HC + 4:HC + 8], start=True, stop=True)
                        # logits l = es + ed_bc ; p = exp(lrelu(l) - ln64)
                        lg = pm.tile([128, cpb, 4], F32, tag="lg")
                        nc.vector.tensor_tensor(
                            lg[:], srcg[:, :, HC:HC + 4], ps_ed[:], OP.add)
                        lr = pm.tile([128, cpb, 4], F32, tag="lr")
                        nc.scalar.activation(lr[:], lg[:], AF.Prelu, alpha=SLOPE)
                        pf = pm.tile([128, cpb, 4], F32, tag="pf")
                        nc.scalar.activation(pf[:], lr[:], AF.Exp, bias=lnb_t[:])
                        p16 = srcg[:, :, HC + 4:HC + 8]
                        nc.scalar.activation(p16, pf[:], AF.Copy)

                        if debug and b == 0 and d == 0:
                            nc.sync.dma_start(dbg_p[:], p16[:].rearrange("p a b -> p (a b)"))
                            ped_sb = pm.tile([128, cpb, 4], F32, tag="pedsb")
                            nc.vector.tensor_copy(ped_sb[:], ps_ed[:])
                            nc.sync.dma_start(dbg_ped[:], ped_sb[:].rearrange("p a b -> p (a b)"))
                        # rows *= p (per-head broadcast multiply)
                        for c in range(cpb):
                            v = srcg[:, c, 0:HC].rearrange("p (h w) -> p h w", w=C)
                            nc.vector.tensor_tensor(
                                v, v,
                                srcg[:, c, HC + 4:HC + 8].unsqueeze(2)
                                .to_broadcast((128, H, C)),
                                OP.mult)

                        if debug and b == 0 and d == 0:
                            nc.sync.dma_start(dbg_srcg2[:], srcg[:].rearrange("p a b -> p (a b)"))
                        # num/den accumulation
                        pnd = psn.tile([128, HC + 8], F32, tag="pnd")
                        for c in range(cpb):
                            mc = mk[:, 0, c * 128:(c + 1) * 128]
                            nc.tensor.matmul(pnd[:, 0:HC + 8], mc,
                                             srcg[:, c, 0:HC + 8],
                                             start=(c == 0), stop=(c == cpb - 1))

                        if debug and b == 0 and d == 0:
                            pnd_sb = pm.tile([128, HC + 4], F32, tag="pndsb")
                            nc.vector.tensor_copy(pnd_sb[:, 0:HC], pnd[:, 0:HC])
                            nc.vector.tensor_copy(pnd_sb[:, HC:], pnd[:, HC + 4:HC + 8])
                            nc.sync.dma_start(dbg_pnd[:], pnd_sb[:])
                        # self-loop p
                        sl = pm.tile([128, 4], F32, tag="sl")
                        nc.vector.tensor_tensor(
                            sl[:], lc[:, HC:HC + 4], lc[:, HC + 4:HC + 8], OP.add)
                        slr = pm.tile([128, 4], F32, tag="slr")
                        nc.scalar.activation(slr[:], sl[:], AF.Prelu, alpha=SLOPE)
                        psf = pm.tile([128, 4], F32, tag="psf")
                        nc.scalar.activation(psf[:], slr[:], AF.Exp, bias=lnb_t[:])

                        # normalize: stage = (num + p_self*h_loc) / (2*(den+p_self))
                        dtot = pm.tile([128, 4], F32, tag="dtot")
                        nc.vector.tensor_tensor(dtot[:], pnd[:, HC + 4:HC + 8], psf[:],
                                                OP.add)
                        nc.vector.tensor_scalar(
                            out=dtot[:], in0=dtot[:], scalar1=2.0, scalar2=1e-30,
                            op0=OP.mult, op1=OP.max)
                        rec = pm.tile([128, 4], F32, tag="rec")
                        nc.vector.reciprocal(rec[:], dtot[:])

                        stg = po.tile([128, H, C], F32, tag=f"stg{d}", name=f"stg{d}")
                        for h in range(H):
                            nc.scalar.activation(
                                stg[:, h, :], lc[:, h * C:(h + 1) * C], AF.Copy,
                                scale=psf[:, h:h + 1])
                        nc.vector.tensor_tensor(
                            stg[:], stg[:],
                            pnd[:, 0:HC].rearrange("p (h w) -> p h w", w=C), OP.add)
                        for h in range(H):
                            nc.scalar.activation(
                                stg[:, h, :], stg[:, h, :], AF.Copy,
                                scale=rec[:, h:h + 1])
                        stage[d] = stg

                    ot = po.tile([128, HC], F32, tag="ot")
                    nc.vector.tensor_tensor(
                        ot[:].rearrange("p (h w) -> p h w", w=C),
                        stage[0][:], stage[1][:], OP.add)
                    nc.vector.tensor_tensor(ot[:], ot[:], bias_bc[:], OP.add)
                    nc.sync.dma_start(out[b * 128:(b + 1) * 128, :], ot[:])

    nc.compile()
    return nc


# ---------------------------------------------------------------- host side

def _wrap16(arr):
    """int idx array [n] -> dma_gather layout [128, n/16] int16 (replicated)."""
    n = len(arr)
    m = arr.reshape(n // 16, 16).astype(np.int16).T  # [16, n/16]
    return np.tile(m, (8, 1))


def prep_inputs(x, edge_index, W1, a_src1, a_dst1, b1, W2, a_src2, a_dst2, b2,
                nbin=NBIN, cb=CB):
    cpb = 2 * cb
    x = np.asarray(x, np.float32)
    ei = np.asarray(edge_index)
    src, dst = ei[0].astype(np.int64), ei[1].astype(np.int64)
    dirs = [(src, dst), (dst, src)]   # no self-loops; handled via local path

    # per-node degree by (dir, src-bank)
    deg = np.zeros((N, 4), np.int64)
    for j, (ss, dd) in enumerate(dirs):
        for bk in range(2):
            m = (ss >= B0REAL) == (bk == 1)
            deg[:, 2 * j + bk] = np.bincount(dd[m], minlength=N)

    # fp16 feature layouts
    x16 = x.astype(np.float16)
    xpad = np.zeros((NT * 128, DIN), np.float16)
    xpad[0:B0REAL] = x16[0:B0REAL]
    xpad[BKROWS:BKROWS + (N - B0REAL)] = x16[B0REAL:N]
    xTb = np.ascontiguousarray(xpad.T.reshape(2, 128, NT * 128))

    W_l = [np.asarray(W1, np.float32), np.asarray(W2, np.float32)]
    a_l = [(np.asarray(a_src1, np.float32), np.asarray(a_dst1, np.float32)),
           (np.asarray(a_src2, np.float32), np.asarray(a_dst2, np.float32))]
    cols = []
    for d in range(2):
        for a in a_l[d]:
            A = np.zeros((HC, H), np.float32)
            for h in range(H):
                A[h * C:(h + 1) * C, h] = a[h]
            cols.append(W_l[d] @ A)         # [256, 4]
    Wsb = np.zeros((2, 2, 128, HC + 8), np.float16)
    for d in range(2):
        wext = np.concatenate([W_l[d], cols[2 * d], cols[2 * d + 1]],
                              axis=1).astype(np.float16)  # [256, 264]
        for k in range(2):
            Wsb[d, k] = wext[k * 128:(k + 1) * 128, :]
    b_in = (0.5 * (np.asarray(b1) + np.asarray(b2))).astype(np.float32)
    b_in = b_in.reshape(1, HC)

    in_maps, perms = [], []
    for core in range(NCORES):
        lo = core * NPC
        nodes = np.arange(lo, lo + NPC)
        order = nodes[np.argsort(-deg[nodes].sum(1), kind="stable")]
        degs = deg[order]
        bins_load = np.zeros((nbin, 4), np.int64)
        bins_cnt = np.zeros(nbin, np.int64)
        node_blk = np.full(N, -1, np.int64)
        node_slot = np.full(N, -1, np.int64)
        for i_n in range(len(order)):
            dgl = degs[i_n]
            ok = (bins_cnt < 128) & ((bins_load + dgl) <= cb * 128).all(1)
            assert ok.any(), "bin packing failed; raise nbin/cb"
            cand = np.where(ok)[0]
            nl = (bins_load[cand] + dgl).max(1) * 1000 + bins_cnt[cand]
            i = cand[np.argmin(nl)]
            node_blk[order[i_n]] = i
            node_slot[order[i_n]] = bins_cnt[i]
            bins_load[i] += dgl
            bins_cnt[i] += 1

        perm = np.full(nbin * 128, -1, np.int64)
        perm[node_blk[nodes] * 128 + node_slot[nodes]] = nodes
        perms.append(perm)

        g_idx = np.zeros((2, nbin, 128, 16 * cb), np.int16)
        m_host = np.zeros((2, nbin, 2, 128, cpb * 128), np.float16)
        for d, (ss, dd) in enumerate(dirs):
            sel = (dd >= lo) & (dd < lo + NPC)
            es_, ed_ = ss[sel], dd[sel]
            blk = node_blk[ed_]
            bank = (es_ >= B0REAL).astype(np.int64)
            eo = np.lexsort((bank, blk))
            es_, ed_, blk, bank = es_[eo], ed_[eo], blk[eo], bank[eo]
            seg = blk * 2 + bank
            segbnd = np.flatnonzero(np.diff(seg, prepend=-1))
            within = np.arange(len(seg)) - np.repeat(segbnd, np.diff(
                np.append(segbnd, len(seg))))
            assert (within < cb * 128).all()
            slot = within + bank * (cb * 128)
            srcrel = np.where(bank == 0, es_, es_ - B0REAL)
            dslot = node_slot[ed_]
            s_idx = np.full((nbin, cpb * 128), PADIDX, np.int64)
            s_idx[blk, slot] = srcrel
            chunk, epart = slot // 128, slot % 128
            m_host[d, blk, 0, epart, chunk * 128 + dslot] = 1.0
            m_host[d, blk, 1, dslot, chunk * 128 + epart] = 1.0
            for bb in range(nbin):
                g_idx[d, bb, :, 0:8 * cb] = _wrap16(s_idx[bb, 0:cb * 128])
                g_idx[d, bb, :, 8 * cb:16 * cb] = _wrap16(s_idx[bb, cb * 128:])

        xloc = np.zeros((nbin * 128, DIN), np.float16)
        valid = perm >= 0
        xloc[valid] = x16[perm[valid]]
        xTl = np.ascontiguousarray(xloc.T.reshape(2, 128, nbin * 128))

        in_maps.append({
            "xTb": xTb, "xTl": xTl, "Wsb": Wsb, "b_in": b_in,
            "gidx": g_idx, "msk": m_host,
        })
    return in_maps, perms


_NC_CACHE = {}


def kernel(**inputs):
    last_err = None
    for nbin, cb in ((NBIN, CB), (53, 4), (51, 5), (55, 5)):
        try:
            in_maps, perms = prep_inputs(**inputs, nbin=nbin, cb=cb)
            break
        except AssertionError as e:
            last_err = e
    else:
        raise last_err
    key = (nbin, cb)
    if key not in _NC_CACHE:
        _NC_CACHE[key] = build_kernel(nbin, cb)
    nc = _NC_CACHE[key]
    res = run_bass_kernel_spmd(nc, in_maps, list(range(NCORES)))
    result = np.empty((N, HC), np.float32)
    for core in range(NCORES):
        o = res.results[core]["out"]
        p = perms[core]
        valid = p >= 0
        result[p[valid]] = o[valid]
    return result


# revision 17
# speedup vs baseline: 3.2782x; 1.0139x over previous
"""DirGATConv on 8 Trainium2 NeuronCores (Bass/Tile), v2.

Strategy (node/data parallel, no collectives):
  - Each core owns 6250 destination nodes, permuted into NBIN blocks of <=128
    by bin packing so every (block, direction, src-bank) has at most CB*128
    non-self-loop edges.
  - Phase A (replicated on every core): h = x @ W_d for all nodes plus the
    per-node attention projections es/ed = x @ (W_d a_*), written to two DRAM
    gather tables per direction (fp16 rows: 256 h | 4 es | 124 pad = 768 B;
    row count per bank <= 32767 because dma_gather indices are int16).  A
    bin-permuted local table per direction holds (h | es | ed) for the core's
    own destinations (544 B rows, read linearly in Phase B).
  - Phase B per (block, direction): dma_gather the source rows (one gather
    per src-bank), then with host-shipped 0/1 fp16 masks M [e,d] / MT [d,e]:
      ed_bc  = MT^T @ ed_tile                    (per-edge dst projection)
      p      = exp(lrelu(es + ed_bc) - ln 64)    (scalar engine; -ln64 keeps
                                                  h*p inside fp16 range)
      rows  *= p (per-head broadcast multiply), then one matmul per chunk
      num    = M^T @ rows, den = M^T @ p         (same stationary mask)
      out_d  = (num + p_self*h_loc) / (2*(den + p_self))
    Softmax normalization is exact because num and den are linear in p and
    any per-edge common factor (the -ln64 bias) cancels in num/den.
  - Host work is graph-structure-only (bin packing, gather indices, masks,
    layout transposes) plus standard weight fusion (W @ a projections).
"""

import numpy as np

import concourse.bacc as bacc
import concourse.mybir as mybir
import concourse.tile as tile
from concourse.bass_utils import run_bass_kernel_spmd
from concourse import library_config

# problem constants
N, E, DIN, H, C = 50000, 400000, 256, 4, 64
HC = H * C
ALPHA, SLOPE = 0.5, 0.2

# distribution constants
NCORES = 8
NPC = N // NCORES              # 6250 destinations per core
B0REAL = 24960                 # real nodes in bank 0 (nodes 0..24959)
BKROWS = 25088                 # rows per table bank (includes zero pad rows)
PADIDX = BKROWS - 1            # gather index for empty edge slots (zero row)
NT = 392                       # main node tiles (2 banks x 196)
NBIN = 50                      # destination blocks per core
CB = 4                         # gather chunks per (block, src-bank)
CPB = 2 * CB                   # chunks per block
NLOC = NBIN * 128
TW = 384                       # table row width (fp16) = 768 B
LW = 272                       # local row width (fp16) = 544 B
LNB = float(np.log(64.0))      # exp bias, cancels in num/den
F16 = mybir.dt.float16
F32 = mybir.dt.float32
I16 = mybir.dt.int16
AF = mybir.ActivationFunctionType
OP = mybir.AluOpType


def build_kernel(nbin=NBIN, cb=CB, debug=False):
    cpb = 2 * cb
    nc = bacc.Bacc("TRN2", num_swdge_queues=4)
    if debug:
        dbg_srcg = nc.dram_tensor("dbg_srcg", [128, cpb * TW], F16, kind="ExternalOutput")
        dbg_ped = nc.dram_tensor("dbg_ped", [128, cpb * 4], F32, kind="ExternalOutput")
        dbg_p = nc.dram_tensor("dbg_p", [128, cpb * 4], F16, kind="ExternalOutput")
        dbg_pnd = nc.dram_tensor("dbg_pnd", [128, HC + 4], F32, kind="ExternalOutput")
        dbg_lc = nc.dram_tensor("dbg_lc", [128, LW], F16, kind="ExternalOutput")
        dbg_srcg2 = nc.dram_tensor("dbg_srcg2", [128, cpb * TW], F16, kind="ExternalOutput")

    xTb = nc.dram_tensor("xTb", [2, 128, NT * 128], F16, kind="ExternalInput")
    xTl = nc.dram_tensor("xTl", [2, 128, nbin * 128], F16, kind="ExternalInput")
    Wsb = nc.dram_tensor("Wsb", [2, 2, 128, HC + 8], F16, kind="ExternalInput")
    b_in = nc.dram_tensor("b_in", [1, HC], F32, kind="ExternalInput")
    gidx = nc.dram_tensor("gidx", [2, nbin, 128, 16 * cb], I16, kind="ExternalInput")
    msk = nc.dram_tensor("msk", [2, nbin, 2, 128, cpb * 128], F16, kind="ExternalInput")
    out = nc.dram_tensor("out", [nbin * 128, HC], F32, kind="ExternalOutput")

    with tile.TileContext(nc) as tc:
        with (
            tc.tile_pool(name="dram", bufs=1, space="DRAM") as dpool,
            tc.tile_pool(name="const", bufs=1) as cpool,
        ):
            nc.gpsimd.load_library(library_config.mlp)

            tabs = [
                [dpool.tile([BKROWS, TW], F16, tag=f"tab{d}{k}", name=f"tab{d}{k}")
                 for k in range(2)]
                for d in range(2)
            ]
            locs = [dpool.tile([nbin * 128, LW], F16, tag=f"loc{d}", name=f"loc{d}")
                    for d in range(2)]

            # weights: w_sb[d] [128 din, 2 k, 264 = hc|es|ed]
            w_sb = [cpool.tile([128, 2, HC + 8], F16, tag=f"w{d}", name=f"w{d}")
                    for d in range(2)]
            for d in range(2):
                nc.sync.dma_start(
                    w_sb[d][:], Wsb[d].rearrange("k p c -> p k c"))
            bias_bc = cpool.tile([128, HC], F32)
            nc.sync.dma_start(bias_bc[:], b_in[:].to_broadcast((128, HC)))
            lnb_t = cpool.tile([128, 1], F32)
            nc.vector.memset(lnb_t[:], -LNB)

            # ---------------- Phase A ----------------
            with (
                tc.tile_pool(name="pAx", bufs=4) as pax,
                tc.tile_pool(name="pAs", bufs=4) as pas,
                tc.tile_pool(name="psA", bufs=4, space="PSUM") as psa,
            ):
                st = [None, None]

                def node_tile(xt_k, j, wide):
                    """One 128-node tile: xt_k [128, 2, 128]; write into
                    st[d][:, j, :] (wide=TW) or st[d] [128, LW] (wide=LW)."""
                    ph0 = psa.tile([128, HC + 8], F32, tag="ph0")
                    ph1 = psa.tile([128, HC + 8], F32, tag="ph1")
                    ph = [ph0, ph1]
                    for k in range(2):
                        for d in range(2):
                            nc.tensor.matmul(
                                ph[d][:], xt_k[:, k, :], w_sb[d][:, k, :],
                                start=(k == 0), stop=(k == 1))
                    if wide == TW:
                        nc.vector.tensor_copy(st[0][:, j, 0:HC + 8], ph[0][:])
                        nc.scalar.activation(st[1][:, j, 0:HC + 8], ph[1][:], AF.Copy)
                    else:
                        nc.vector.tensor_copy(st[0][:, 0:HC + 8], ph[0][:])
                        nc.scalar.activation(st[1][:, 0:HC + 8], ph[1][:], AF.Copy)

                # main tiles: 4-tile batches (bank boundary at tile 196 = 49*4)
                for it in range(NT // 4):
                    xt = pax.tile([128, 4, 2, 128], F16, tag="xt")
                    for k in range(2):
                        nc.sync.dma_start(
                            xt[:, :, k, :],
                            xTb[k, :, it * 512:(it + 1) * 512].rearrange(
                                "p (t c) -> p t c", c=128))
                    for d in range(2):
                        st[d] = pas.tile([128, 4, TW], F16, tag=f"st{d}", name=f"st{d}")
                    for t in range(4):
                        node_tile(xt[:, t, :, :], t, TW)
                    t0 = it * 4
                    bk = 0 if t0 < 196 else 1
                    r0 = (t0 - (0 if bk == 0 else 196)) * 128
                    for d in range(2):
                        dst = tabs[d][bk][r0:r0 + 512, :].rearrange(
                            "(t p) c -> p t c", t=4)
                        eng = nc.gpsimd if d == 0 else nc.scalar
                        eng.dma_start(dst, st[d][:])

                # local tiles (one per iteration)
                for t in range(nbin):
                    xt = pax.tile([128, 1, 2, 128], F16, tag="xt")
                    for k in range(2):
                        nc.sync.dma_start(
                            xt[:, 0, k, :],
                            xTl[k, :, t * 128:(t + 1) * 128])
                    for d in range(2):
                        st[d] = pas.tile([128, LW], F16, tag=f"lst{d}", name=f"lst{d}")
                    node_tile(xt[:, 0, :, :], 0, LW)
                    for d in range(2):
                        eng = nc.gpsimd if d == 0 else nc.scalar
                        eng.dma_start(locs[d][t * 128:(t + 1) * 128, :], st[d][:])

            # ---------------- Phase B ----------------
            with (
                tc.tile_pool(name="pBg", bufs=5) as pg,
                tc.tile_pool(name="pBk", bufs=4) as pk,
                tc.tile_pool(name="pBm", bufs=6) as pm,
                tc.tile_pool(name="pBo", bufs=2) as po,
                tc.tile_pool(name="psN", bufs=3, space="PSUM") as psn,
                tc.tile_pool(name="psE", bufs=2, space="PSUM") as pse,
            ):
                for b in range(nbin):
                    stage = [None, None]
                    for d in range(2):
                        gi = pm.tile([128, 16 * cb], I16, tag="gi")
                        nc.sync.dma_start(gi[:], gidx[d, b])
                        mk = pk.tile([128, 2, cpb * 128], F16, tag="mk")
                        nc.scalar.dma_start(mk[:], msk[d, b].rearrange("m p c -> p m c"))
                        lc = pm.tile([128, LW], F16, tag="lc")
                        nc.sync.dma_start(lc[:], locs[d][b * 128:(b + 1) * 128, :])

                        srcg = pg.tile([128, cpb, TW], F16, tag="srcg")
                        for half in range(2):
                            nc.gpsimd.dma_gather(
                                srcg[:, half * cb:(half + 1) * cb, :],
                                tabs[d][half][:],
                                gi[:, half * 8 * cb:(half + 1) * 8 * cb],
                                cb * 128, cb * 128, TW,
                                queue_num=(2 * (2 * b + d) + half) % 4,
                                single_packet=False)

                        if debug and b == 0 and d == 0:
                            nc.sync.dma_start(dbg_srcg[:], srcg[:].rearrange("p a b -> p (a b)"))
                            nc.sync.dma_start(dbg_lc[:], lc[:])
                        # ed_bc[e, h] via MT^T @ ed_tile
                        ps_ed = pse.tile([128, cpb, 4], F32, tag="ped")
                        for c in range(cpb):
                            nc.tensor.matmul(
                                ps_ed[:, c, :], mk[:, 1, c * 128:(c + 1) * 128],
                                lc[:, HC + 4:HC + 8], start=True, stop=True)
                        # logits l = es + ed_bc ; p = exp(lrelu(l) - ln64)
                        lg = pm.tile([128, cpb, 4], F32, tag="lg")
                        nc.vector.tensor_tensor(
                            lg[:], srcg[:, :, HC:HC + 4], ps_ed[:], OP.add)
                        lr = pm.tile([128, cpb, 4], F32, tag="lr")
                        nc.scalar.activation(lr[:], lg[:], AF.Prelu, alpha=SLOPE)
                        pf = pm.tile([128, cpb, 4], F32, tag="pf")
                        nc.scalar.activation(pf[:], lr[:], AF.Exp, bias=lnb_t[:])
                        p16 = srcg[:, :, HC + 4:HC + 8]
                        nc.scalar.activation(p16, pf[:], AF.Copy)

                        if debug and b == 0 and d == 0:
                            nc.sync.dma_start(dbg_p[:], p16[:].rearrange("p a b -> p (a b)"))
                            ped_sb = pm.tile([128, cpb, 4], F32, tag="pedsb")
                            nc.vector.tensor_copy(ped_sb[:], ps_ed[:])
                            nc.sync.dma_start(dbg_ped[:], ped_sb[:].rearrange("p a b -> p (a b)"))
                        # rows *= p (per-head broadcast multiply)
                        for c in range(cpb):
                            v = srcg[:, c, 0:HC].rearrange("p (h w) -> p h w", w=C)
                            nc.vector.tensor_tensor(
                                v, v,
                                srcg[:, c, HC + 4:HC + 8].unsqueeze(2)
                                .to_broadcast((128, H, C)),
                                OP.mult)

                        if debug and b == 0 and d == 0:
                            nc.sync.dma_start(dbg_srcg2[:], srcg[:].rearrange("p a b -> p (a b)"))
                        # num/den accumulation
                        pnd = psn.tile([128, HC + 8], F32, tag="pnd")
                        for c in range(cpb):
                            mc = mk[:, 0, c * 128:(c + 1) * 128]
                            nc.tensor.matmul(pnd[:, 0:HC + 8], mc,
                                             srcg[:, c, 0:HC + 8],
                                             start=(c == 0), stop=(c == cpb - 1))

                        if debug and b == 0 and d == 0:
                            pnd_sb = pm.tile([128, HC + 4], F32, tag="pndsb")
                            nc.vector.tensor_copy(pnd_sb[:, 0:HC], pnd[:, 0:HC])
                            nc.vector.tensor_copy(pnd_sb[:, HC:], pnd[:, HC + 4:HC + 8])
                            nc.sync.dma_start(dbg_pnd[:], pnd_sb[:])
                        # self-loop p
                        sl = pm.tile([128, 4], F32, tag="sl")
                        nc.vector.tensor_tensor(
                            sl[:], lc[:, HC:HC + 4], lc[:, HC + 4:HC + 8], OP.add)
                        slr = pm.tile([128, 4], F32, tag="slr")
                        nc.scalar.activation(slr[:], sl[:], AF.Prelu, alpha=SLOPE)
                        psf = pm.tile([128, 4], F32, tag="psf")
                        nc.scalar.activation(psf[:], slr[:], AF.Exp, bias=lnb_t[:])

                        # normalize: stage = (num + p_self*h_loc) / (2*(den+p_self))
                        dtot = pm.tile([128, 4], F32, tag="dtot")
                        nc.vector.tensor_tensor(dtot[:], pnd[:, HC + 4:HC + 8], psf[:],
                                                OP.add)
                        nc.vector.tensor_scalar(
                            out=dtot[:], in0=dtot[:], scalar1=2.0, scalar2=1e-30,
                            op0=OP.mult, op1=OP.max)
                        rec = pm.tile([128, 4], F32, tag="rec")
                        nc.vector.reciprocal(rec[:], dtot[:])

                        stg = po.tile([128, H, C], F32, tag=f"stg{d}", name=f"stg{d}")
                        for h in range(H):
                            nc.scalar.activation(
                                stg[:, h, :], lc[:, h * C:(h + 1) * C], AF.Copy,
                                scale=psf[:, h:h + 1])
                        nc.vector.tensor_tensor(
                            stg[:], stg[:],
                            pnd[:, 0:HC].rearrange("p (h w) -> p h w", w=C), OP.add)
                        for h in range(H):
                            nc.scalar.activation(
                                stg[:, h, :], stg[:, h, :], AF.Copy,
                                scale=rec[:, h:h + 1])
                        stage[d] = stg

                    ot = po.tile([128, HC], F32, tag="ot")
                    nc.vector.tensor_tensor(
                        ot[:].rearrange("p (h w) -> p h w", w=C),
                        stage[0][:], stage[1][:], OP.add)
                    nc.vector.tensor_tensor(ot[:], ot[:], bias_bc[:], OP.add)
                    nc.sync.dma_start(out[b * 128:(b + 1) * 128, :], ot[:])

    nc.compile()
    return nc


# ---------------------------------------------------------------- host side

def _wrap16(arr):
    """int idx array [n] -> dma_gather layout [128, n/16] int16 (replicated)."""
    n = len(arr)
    m = arr.reshape(n // 16, 16).astype(np.int16).T  # [16, n/16]
    return np.tile(m, (8, 1))


def prep_inputs(x, edge_index, W1, a_src1, a_dst1, b1, W2, a_src2, a_dst2, b2,
                nbin=NBIN, cb=CB):
    cpb = 2 * cb
    x = np.asarray(x, np.float32)
    ei = np.asarray(edge_index)
    src, dst = ei[0].astype(np.int64), ei[1].astype(np.int64)
    dirs = [(src, dst), (dst, src)]   # no self-loops; handled via local path

    # per-node degree by (dir, src-bank)
    deg = np.zeros((N, 4), np.int64)
    for j, (ss, dd) in enumerate(dirs):
        for bk in range(2):
            m = (ss >= B0REAL) == (bk == 1)
            deg[:, 2 * j + bk] = np.bincount(dd[m], minlength=N)

    # fp16 feature layouts
    x16 = x.astype(np.float16)
    xpad = np.zeros((NT * 128, DIN), np.float16)
    xpad[0:B0REAL] = x16[0:B0REAL]
    xpad[BKROWS:BKROWS + (N - B0REAL)] = x16[B0REAL:N]
    xTb = np.ascontiguousarray(xpad.T.reshape(2, 128, NT * 128))

    W_l = [np.asarray(W1, np.float32), np.asarray(W2, np.float32)]
    a_l = [(np.asarray(a_src1, np.float32), np.asarray(a_dst1, np.float32)),
           (np.asarray(a_src2, np.float32), np.asarray(a_dst2, np.float32))]
    cols = []
    for d in range(2):
        for a in a_l[d]:
            A = np.zeros((HC, H), np.float32)
            for h in range(H):
                A[h * C:(h + 1) * C, h] = a[h]
            cols.append(W_l[d] @ A)         # [256, 4]
    Wsb = np.zeros((2, 2, 128, HC + 8), np.float16)
    for d in range(2):
        wext = np.concatenate([W_l[d], cols[2 * d], cols[2 * d + 1]],
                              axis=1).astype(np.float16)  # [256, 264]
        for k in range(2):
            Wsb[d, k] = wext[k * 128:(k + 1) * 128, :]
    b_in = (0.5 * (np.asarray(b1) + np.asarray(b2))).astype(np.float32)
    b_in = b_in.reshape(1, HC)

    in_maps, perms = [], []
    for core in range(NCORES):
        lo = core * NPC
        nodes = np.arange(lo, lo + NPC)
        order = nodes[np.argsort(-deg[nodes].sum(1), kind="stable")]
        degs = deg[order]
        bins_load = np.zeros((nbin, 4), np.int64)
        bins_cnt = np.zeros(nbin, np.int64)
        node_blk = np.full(N, -1, np.int64)
        node_slot = np.full(N, -1, np.int64)
        for i_n in range(len(order)):
            dgl = degs[i_n]
            ok = (bins_cnt < 128) & ((bins_load + dgl) <= cb * 128).all(1)
            assert ok.any(), "bin packing failed; raise nbin/cb"
            cand = np.where(ok)[0]
            nl = (bins_load[cand] + dgl).max(1) * 1000 + bins_cnt[cand]
            i = cand[np.argmin(nl)]
            node_blk[order[i_n]] = i
            node_slot[order[i_n]] = bins_cnt[i]
            bins_load[i] += dgl
            bins_cnt[i] += 1

        perm = np.full(nbin * 128, -1, np.int64)
        perm[node_blk[nodes] * 128 + node_slot[nodes]] = nodes
        perms.append(perm)

        g_idx = np.zeros((2, nbin, 128, 16 * cb), np.int16)
        m_host = np.zeros((2, nbin, 2, 128, cpb * 128), np.float16)
        for d, (ss, dd) in enumerate(dirs):
            sel = (dd >= lo) & (dd < lo + NPC)
            es_, ed_ = ss[sel], dd[sel]
            blk = node_blk[ed_]
            bank = (es_ >= B0REAL).astype(np.int64)
            eo = np.lexsort((bank, blk))
            es_, ed_, blk, bank = es_[eo], ed_[eo], blk[eo], bank[eo]
            seg = blk * 2 + bank
            segbnd = np.flatnonzero(np.diff(seg, prepend=-1))
            within = np.arange(len(seg)) - np.repeat(segbnd, np.diff(
                np.append(segbnd, len(seg))))
            assert (within < cb * 128).all()
            slot = within + bank * (cb * 128)
            srcrel = np.where(bank == 0, es_, es_ - B0REAL)
            dslot = node_slot[ed_]
            s_idx = np.full((nbin, cpb * 128), PADIDX, np.int64)
            s_idx[blk, slot] = srcrel
            chunk, epart = slot // 128, slot % 128
            m_host[d, blk, 0, epart, chunk * 128 + dslot] = 1.0
            m_host[d, blk, 1, dslot, chunk * 128 + epart] = 1.0
            for bb in range(nbin):
                g_idx[d, bb, :, 0:8 * cb] = _wrap16(s_idx[bb, 0:cb * 128])
                g_idx[d, bb, :, 8 * cb:16 * cb] = _wrap16(s_idx[bb, cb * 128:])

        xloc = np.zeros((nbin * 128, DIN), np.float16)
        valid = perm >= 0
        xloc[valid] = x16[perm[valid]]
        xTl = np.ascontiguousarray(xloc.T.reshape(2, 128, nbin * 128))

        in_maps.append({
            "xTb": xTb, "xTl": xTl, "Wsb": Wsb, "b_in": b_in,
            "gidx": g_idx, "msk": m_host,
        })
    return in_maps, perms


_NC_CACHE = {}


def kernel(**inputs):
    last_err = None
    for nbin, cb in ((NBIN, CB), (53, 4), (51, 5), (55, 5)):
        try:
            in_maps, perms = prep_inputs(**inputs, nbin=nbin, cb=cb)
            break
        except AssertionError as e:
            last_err = e
    else:
        raise last_err
    key = (nbin, cb)
    if key not in _NC_CACHE:
        _NC_CACHE[key] = build_kernel(nbin, cb)
    nc = _NC_CACHE[key]
    res = run_bass_kernel_spmd(nc, in_maps, list(range(NCORES)))
    result = np.empty((N, HC), np.float32)
    for core in range(NCORES):
        o = res.results[core]["out"]
        p = perms[core]
        valid = p >= 0
        result[p[valid]] = o[valid]
    return result


# revision 18
# speedup vs baseline: 3.2936x; 1.0047x over previous
"""DirGATConv on 8 Trainium2 NeuronCores (Bass/Tile), v2.

Strategy (node/data parallel, no collectives):
  - Each core owns 6250 destination nodes, permuted into NBIN blocks of <=128
    by bin packing so every (block, direction, src-bank) has at most CB*128
    non-self-loop edges.
  - Phase A (replicated on every core): h = x @ W_d for all nodes plus the
    per-node attention projections es/ed = x @ (W_d a_*), written to two DRAM
    gather tables per direction (fp16 rows: 256 h | 4 es | 124 pad = 768 B;
    row count per bank <= 32767 because dma_gather indices are int16).  A
    bin-permuted local table per direction holds (h | es | ed) for the core's
    own destinations (544 B rows, read linearly in Phase B).
  - Phase B per (block, direction): dma_gather the source rows (one gather
    per src-bank), then with host-shipped 0/1 fp16 masks M [e,d] / MT [d,e]:
      ed_bc  = MT^T @ ed_tile                    (per-edge dst projection)
      p      = exp(lrelu(es + ed_bc) - ln 64)    (scalar engine; -ln64 keeps
                                                  h*p inside fp16 range)
      rows  *= p (per-head broadcast multiply), then one matmul per chunk
      num    = M^T @ rows, den = M^T @ p         (same stationary mask)
      out_d  = (num + p_self*h_loc) / (2*(den + p_self))
    Softmax normalization is exact because num and den are linear in p and
    any per-edge common factor (the -ln64 bias) cancels in num/den.
  - Host work is graph-structure-only (bin packing, gather indices, masks,
    layout transposes) plus standard weight fusion (W @ a projections).
"""

import numpy as np

import concourse.bacc as bacc
import concourse.mybir as mybir
import concourse.tile as tile
from concourse.bass_utils import run_bass_kernel_spmd
from concourse import library_config

# problem constants
N, E, DIN, H, C = 50000, 400000, 256, 4, 64
HC = H * C
ALPHA, SLOPE = 0.5, 0.2

# distribution constants
NCORES = 8
NPC = N // NCORES              # 6250 destinations per core
B0REAL = 24960                 # real nodes in bank 0 (nodes 0..24959)
BKROWS = 25088                 # rows per table bank (includes zero pad rows)
PADIDX = BKROWS - 1            # gather index for empty edge slots (zero row)
NT = 392                       # main node tiles (2 banks x 196)
NBIN = 50                      # destination blocks per core
CB = 4                         # gather chunks per (block, src-bank)
CPB = 2 * CB                   # chunks per block
NLOC = NBIN * 128
TW = 384                       # table row width (fp16) = 768 B
LW = 272                       # local row width (fp16) = 544 B
LNB = float(np.log(64.0))      # exp bias, cancels in num/den
F16 = mybir.dt.float16
F32 = mybir.dt.float32
I16 = mybir.dt.int16
AF = mybir.ActivationFunctionType
OP = mybir.AluOpType


def build_kernel(nbin=NBIN, cb=CB, debug=False):
    cpb = 2 * cb
    nc = bacc.Bacc("TRN2", num_swdge_queues=4)
    if debug:
        dbg_srcg = nc.dram_tensor("dbg_srcg", [128, cpb * TW], F16, kind="ExternalOutput")
        dbg_ped = nc.dram_tensor("dbg_ped", [128, cpb * 4], F32, kind="ExternalOutput")
        dbg_p = nc.dram_tensor("dbg_p", [128, cpb * 4], F16, kind="ExternalOutput")
        dbg_pnd = nc.dram_tensor("dbg_pnd", [128, HC + 4], F32, kind="ExternalOutput")
        dbg_lc = nc.dram_tensor("dbg_lc", [128, LW], F16, kind="ExternalOutput")
        dbg_srcg2 = nc.dram_tensor("dbg_srcg2", [128, cpb * TW], F16, kind="ExternalOutput")

    xTb = nc.dram_tensor("xTb", [2, 128, NT * 128], F16, kind="ExternalInput")
    xTl = nc.dram_tensor("xTl", [2, 128, nbin * 128], F16, kind="ExternalInput")
    Wsb = nc.dram_tensor("Wsb", [2, 2, 128, HC + 8], F16, kind="ExternalInput")
    b_in = nc.dram_tensor("b_in", [1, HC], F32, kind="ExternalInput")
    gidx = nc.dram_tensor("gidx", [2, nbin, 128, 16 * cb], I16, kind="ExternalInput")
    msk = nc.dram_tensor("msk", [2, nbin, 2, 128, cpb * 128], F16, kind="ExternalInput")
    out = nc.dram_tensor("out", [nbin * 128, HC], F32, kind="ExternalOutput")

    with tile.TileContext(nc) as tc:
        with (
            tc.tile_pool(name="dram", bufs=1, space="DRAM") as dpool,
            tc.tile_pool(name="const", bufs=1) as cpool,
        ):
            nc.gpsimd.load_library(library_config.mlp)

            tabs = [
                [dpool.tile([BKROWS, TW], F16, tag=f"tab{d}{k}", name=f"tab{d}{k}")
                 for k in range(2)]
                for d in range(2)
            ]
            locs = [dpool.tile([nbin * 128, LW], F16, tag=f"loc{d}", name=f"loc{d}")
                    for d in range(2)]

            # weights: w_sb[d] [128 din, 2 k, 264 = hc|es|ed]
            w_sb = [cpool.tile([128, 2, HC + 8], F16, tag=f"w{d}", name=f"w{d}")
                    for d in range(2)]
            for d in range(2):
                nc.sync.dma_start(
                    w_sb[d][:], Wsb[d].rearrange("k p c -> p k c"))
            bias_bc = cpool.tile([128, HC], F32)
            nc.sync.dma_start(bias_bc[:], b_in[:].to_broadcast((128, HC)))
            lnb_t = cpool.tile([128, 1], F32)
            nc.vector.memset(lnb_t[:], -LNB)

            # ---------------- Phase A ----------------
            with (
                tc.tile_pool(name="pAx", bufs=8) as pax,
                tc.tile_pool(name="pAs", bufs=6) as pas,
                tc.tile_pool(name="psA", bufs=4, space="PSUM") as psa,
            ):
                st = [None, None]

                def node_tile(xt_k, j, wide):
                    """One 128-node tile: xt_k [128, 2, 128]; write into
                    st[d][:, j, :] (wide=TW) or st[d] [128, LW] (wide=LW)."""
                    ph0 = psa.tile([128, HC + 8], F32, tag="ph0")
                    ph1 = psa.tile([128, HC + 8], F32, tag="ph1")
                    ph = [ph0, ph1]
                    for k in range(2):
                        for d in range(2):
                            nc.tensor.matmul(
                                ph[d][:], xt_k[:, k, :], w_sb[d][:, k, :],
                                start=(k == 0), stop=(k == 1))
                    if wide == TW:
                        nc.vector.tensor_copy(st[0][:, j, 0:HC + 8], ph[0][:])
                        nc.scalar.activation(st[1][:, j, 0:HC + 8], ph[1][:], AF.Copy)
                    else:
                        nc.vector.tensor_copy(st[0][:, 0:HC + 8], ph[0][:])
                        nc.scalar.activation(st[1][:, 0:HC + 8], ph[1][:], AF.Copy)

                # main tiles: 4-tile batches (bank boundary at tile 196 = 49*4)
                for it in range(NT // 4):
                    xt = pax.tile([128, 4, 2, 128], F16, tag="xt")
                    for k in range(2):
                        nc.sync.dma_start(
                            xt[:, :, k, :],
                            xTb[k, :, it * 512:(it + 1) * 512].rearrange(
                                "p (t c) -> p t c", c=128))
                    for d in range(2):
                        st[d] = pas.tile([128, 4, TW], F16, tag=f"st{d}", name=f"st{d}")
                    for t in range(4):
                        node_tile(xt[:, t, :, :], t, TW)
                    t0 = it * 4
                    bk = 0 if t0 < 196 else 1
                    r0 = (t0 - (0 if bk == 0 else 196)) * 128
                    for d in range(2):
                        dst = tabs[d][bk][r0:r0 + 512, :].rearrange(
                            "(t p) c -> p t c", t=4)
                        eng = nc.gpsimd if d == 0 else nc.scalar
                        eng.dma_start(dst, st[d][:])

                # local tiles (one per iteration)
                for t in range(nbin):
                    xt = pax.tile([128, 1, 2, 128], F16, tag="xt")
                    for k in range(2):
                        nc.sync.dma_start(
                            xt[:, 0, k, :],
                            xTl[k, :, t * 128:(t + 1) * 128])
                    for d in range(2):
                        st[d] = pas.tile([128, LW], F16, tag=f"lst{d}", name=f"lst{d}")
                    node_tile(xt[:, 0, :, :], 0, LW)
                    for d in range(2):
                        eng = nc.gpsimd if d == 0 else nc.scalar
                        eng.dma_start(locs[d][t * 128:(t + 1) * 128, :], st[d][:])

            # ---------------- Phase B ----------------
            with (
                tc.tile_pool(name="pBg", bufs=5) as pg,
                tc.tile_pool(name="pBk", bufs=4) as pk,
                tc.tile_pool(name="pBm", bufs=6) as pm,
                tc.tile_pool(name="pBo", bufs=2) as po,
                tc.tile_pool(name="psN", bufs=3, space="PSUM") as psn,
                tc.tile_pool(name="psE", bufs=2, space="PSUM") as pse,
            ):
                for b in range(nbin):
                    stage = [None, None]
                    for d in range(2):
                        gi = pm.tile([128, 16 * cb], I16, tag="gi")
                        nc.sync.dma_start(gi[:], gidx[d, b])
                        mk = pk.tile([128, 2, cpb * 128], F16, tag="mk")
                        nc.scalar.dma_start(mk[:], msk[d, b].rearrange("m p c -> p m c"))
                        lc = pm.tile([128, LW], F16, tag="lc")
                        nc.sync.dma_start(lc[:], locs[d][b * 128:(b + 1) * 128, :])

                        srcg = pg.tile([128, cpb, TW], F16, tag="srcg")
                        for half in range(2):
                            nc.gpsimd.dma_gather(
                                srcg[:, half * cb:(half + 1) * cb, :],
                                tabs[d][half][:],
                                gi[:, half * 8 * cb:(half + 1) * 8 * cb],
                                cb * 128, cb * 128, TW,
                                queue_num=(2 * (2 * b + d) + half) % 4,
                                single_packet=False)

                        if debug and b == 0 and d == 0:
                            nc.sync.dma_start(dbg_srcg[:], srcg[:].rearrange("p a b -> p (a b)"))
                            nc.sync.dma_start(dbg_lc[:], lc[:])
                        # ed_bc[e, h] via MT^T @ ed_tile
                        ps_ed = pse.tile([128, cpb, 4], F32, tag="ped")
                        for c in range(cpb):
                            nc.tensor.matmul(
                                ps_ed[:, c, :], mk[:, 1, c * 128:(c + 1) * 128],
                                lc[:, HC + 4:HC + 8], start=True, stop=True)
                        # logits l = es + ed_bc ; p = exp(lrelu(l) - ln64)
                        lg = pm.tile([128, cpb, 4], F32, tag="lg")
                        nc.vector.tensor_tensor(
                            lg[:], srcg[:, :, HC:HC + 4], ps_ed[:], OP.add)
                        lr = pm.tile([128, cpb, 4], F32, tag="lr")
                        nc.scalar.activation(lr[:], lg[:], AF.Prelu, alpha=SLOPE)
                        pf = pm.tile([128, cpb, 4], F32, tag="pf")
                        nc.scalar.activation(pf[:], lr[:], AF.Exp, bias=lnb_t[:])
                        p16 = srcg[:, :, HC + 4:HC + 8]
                        nc.scalar.activation(p16, pf[:], AF.Copy)

                        if debug and b == 0 and d == 0:
                            nc.sync.dma_start(dbg_p[:], p16[:].rearrange("p a b -> p (a b)"))
                            ped_sb = pm.tile([128, cpb, 4], F32, tag="pedsb")
                            nc.vector.tensor_copy(ped_sb[:], ps_ed[:])
                            nc.sync.dma_start(dbg_ped[:], ped_sb[:].rearrange("p a b -> p (a b)"))
                        # rows *= p (per-head broadcast multiply)
                        for c in range(cpb):
                            v = srcg[:, c, 0:HC].rearrange("p (h w) -> p h w", w=C)
                            nc.vector.tensor_tensor(
                                v, v,
                                srcg[:, c, HC + 4:HC + 8].unsqueeze(2)
                                .to_broadcast((128, H, C)),
                                OP.mult)

                        if debug and b == 0 and d == 0:
                            nc.sync.dma_start(dbg_srcg2[:], srcg[:].rearrange("p a b -> p (a b)"))
                        # num/den accumulation
                        pnd = psn.tile([128, HC + 8], F32, tag="pnd")
                        for c in range(cpb):
                            mc = mk[:, 0, c * 128:(c + 1) * 128]
                            nc.tensor.matmul(pnd[:, 0:HC + 8], mc,
                                             srcg[:, c, 0:HC + 8],
                                             start=(c == 0), stop=(c == cpb - 1))

                        if debug and b == 0 and d == 0:
                            pnd_sb = pm.tile([128, HC + 4], F32, tag="pndsb")
                            nc.vector.tensor_copy(pnd_sb[:, 0:HC], pnd[:, 0:HC])
                            nc.vector.tensor_copy(pnd_sb[:, HC:], pnd[:, HC + 4:HC + 8])
                            nc.sync.dma_start(dbg_pnd[:], pnd_sb[:])
                        # self-loop p
                        sl = pm.tile([128, 4], F32, tag="sl")
                        nc.vector.tensor_tensor(
                            sl[:], lc[:, HC:HC + 4], lc[:, HC + 4:HC + 8], OP.add)
                        slr = pm.tile([128, 4], F32, tag="slr")
                        nc.scalar.activation(slr[:], sl[:], AF.Prelu, alpha=SLOPE)
                        psf = pm.tile([128, 4], F32, tag="psf")
                        nc.scalar.activation(psf[:], slr[:], AF.Exp, bias=lnb_t[:])

                        # normalize: stage = (num + p_self*h_loc) / (2*(den+p_self))
                        dtot = pm.tile([128, 4], F32, tag="dtot")
                        nc.vector.tensor_tensor(dtot[:], pnd[:, HC + 4:HC + 8], psf[:],
                                                OP.add)
                        nc.vector.tensor_scalar(
                            out=dtot[:], in0=dtot[:], scalar1=2.0, scalar2=1e-30,
                            op0=OP.mult, op1=OP.max)
                        rec = pm.tile([128, 4], F32, tag="rec")
                        nc.vector.reciprocal(rec[:], dtot[:])

                        stg = po.tile([128, H, C], F32, tag=f"stg{d}", name=f"stg{d}")
                        for h in range(H):
                            nc.scalar.activation(
                                stg[:, h, :], lc[:, h * C:(h + 1) * C], AF.Copy,
                                scale=psf[:, h:h + 1])
                        nc.vector.tensor_tensor(
                            stg[:], stg[:],
                            pnd[:, 0:HC].rearrange("p (h w) -> p h w", w=C), OP.add)
                        for h in range(H):
                            nc.scalar.activation(
                                stg[:, h, :], stg[:, h, :], AF.Copy,
                                scale=rec[:, h:h + 1])
                        stage[d] = stg

                    ot = po.tile([128, HC], F32, tag="ot")
                    nc.vector.tensor_tensor(
                        ot[:].rearrange("p (h w) -> p h w", w=C),
                        stage[0][:], stage[1][:], OP.add)
                    nc.vector.tensor_tensor(ot[:], ot[:], bias_bc[:], OP.add)
                    nc.sync.dma_start(out[b * 128:(b + 1) * 128, :], ot[:])

    nc.compile()
    return nc


# ---------------------------------------------------------------- host side

def _wrap16(arr):
    """int idx array [n] -> dma_gather layout [128, n/16] int16 (replicated)."""
    n = len(arr)
    m = arr.reshape(n // 16, 16).astype(np.int16).T  # [16, n/16]
    return np.tile(m, (8, 1))


def prep_inputs(x, edge_index, W1, a_src1, a_dst1, b1, W2, a_src2, a_dst2, b2,
                nbin=NBIN, cb=CB):
    cpb = 2 * cb
    x = np.asarray(x, np.float32)
    ei = np.asarray(edge_index)
    src, dst = ei[0].astype(np.int64), ei[1].astype(np.int64)
    dirs = [(src, dst), (dst, src)]   # no self-loops; handled via local path

    # per-node degree by (dir, src-bank)
    deg = np.zeros((N, 4), np.int64)
    for j, (ss, dd) in enumerate(dirs):
        for bk in range(2):
            m = (ss >= B0REAL) == (bk == 1)
            deg[:, 2 * j + bk] = np.bincount(dd[m], minlength=N)

    # fp16 feature layouts
    x16 = x.astype(np.float16)
    xpad = np.zeros((NT * 128, DIN), np.float16)
    xpad[0:B0REAL] = x16[0:B0REAL]
    xpad[BKROWS:BKROWS + (N - B0REAL)] = x16[B0REAL:N]
    xTb = np.ascontiguousarray(xpad.T.reshape(2, 128, NT * 128))

    W_l = [np.asarray(W1, np.float32), np.asarray(W2, np.float32)]
    a_l = [(np.asarray(a_src1, np.float32), np.asarray(a_dst1, np.float32)),
           (np.asarray(a_src2, np.float32), np.asarray(a_dst2, np.float32))]
    cols = []
    for d in range(2):
        for a in a_l[d]:
            A = np.zeros((HC, H), np.float32)
            for h in range(H):
                A[h * C:(h + 1) * C, h] = a[h]
            cols.append(W_l[d] @ A)         # [256, 4]
    Wsb = np.zeros((2, 2, 128, HC + 8), np.float16)
    for d in range(2):
        wext = np.concatenate([W_l[d], cols[2 * d], cols[2 * d + 1]],
                              axis=1).astype(np.float16)  # [256, 264]
        for k in range(2):
            Wsb[d, k] = wext[k * 128:(k + 1) * 128, :]
    b_in = (0.5 * (np.asarray(b1) + np.asarray(b2))).astype(np.float32)
    b_in = b_in.reshape(1, HC)

    in_maps, perms = [], []
    for core in range(NCORES):
        lo = core * NPC
        nodes = np.arange(lo, lo + NPC)
        order = nodes[np.argsort(-deg[nodes].sum(1), kind="stable")]
        degs = deg[order]
        bins_load = np.zeros((nbin, 4), np.int64)
        bins_cnt = np.zeros(nbin, np.int64)
        node_blk = np.full(N, -1, np.int64)
        node_slot = np.full(N, -1, np.int64)
        for i_n in range(len(order)):
            dgl = degs[i_n]
            ok = (bins_cnt < 128) & ((bins_load + dgl) <= cb * 128).all(1)
            assert ok.any(), "bin packing failed; raise nbin/cb"
            cand = np.where(ok)[0]
            nl = (bins_load[cand] + dgl).max(1) * 1000 + bins_cnt[cand]
            i = cand[np.argmin(nl)]
            node_blk[order[i_n]] = i
            node_slot[order[i_n]] = bins_cnt[i]
            bins_load[i] += dgl
            bins_cnt[i] += 1

        perm = np.full(nbin * 128, -1, np.int64)
        perm[node_blk[nodes] * 128 + node_slot[nodes]] = nodes
        perms.append(perm)

        g_idx = np.zeros((2, nbin, 128, 16 * cb), np.int16)
        m_host = np.zeros((2, nbin, 2, 128, cpb * 128), np.float16)
        for d, (ss, dd) in enumerate(dirs):
            sel = (dd >= lo) & (dd < lo + NPC)
            es_, ed_ = ss[sel], dd[sel]
            blk = node_blk[ed_]
            bank = (es_ >= B0REAL).astype(np.int64)
            eo = np.lexsort((bank, blk))
            es_, ed_, blk, bank = es_[eo], ed_[eo], blk[eo], bank[eo]
            seg = blk * 2 + bank
            segbnd = np.flatnonzero(np.diff(seg, prepend=-1))
            within = np.arange(len(seg)) - np.repeat(segbnd, np.diff(
                np.append(segbnd, len(seg))))
            assert (within < cb * 128).all()
            slot = within + bank * (cb * 128)
            srcrel = np.where(bank == 0, es_, es_ - B0REAL)
            dslot = node_slot[ed_]
            s_idx = np.full((nbin, cpb * 128), PADIDX, np.int64)
            s_idx[blk, slot] = srcrel
            chunk, epart = slot // 128, slot % 128
            m_host[d, blk, 0, epart, chunk * 128 + dslot] = 1.0
            m_host[d, blk, 1, dslot, chunk * 128 + epart] = 1.0
            for bb in range(nbin):
                g_idx[d, bb, :, 0:8 * cb] = _wrap16(s_idx[bb, 0:cb * 128])
                g_idx[d, bb, :, 8 * cb:16 * cb] = _wrap16(s_idx[bb, cb * 128:])

        xloc = np.zeros((nbin * 128, DIN), np.float16)
        valid = perm >= 0
        xloc[valid] = x16[perm[valid]]
        xTl = np.ascontiguousarray(xloc.T.reshape(2, 128, nbin * 128))

        in_maps.append({
            "xTb": xTb, "xTl": xTl, "Wsb": Wsb, "b_in": b_in,
            "gidx": g_idx, "msk": m_host,
        })
    return in_maps, perms


_NC_CACHE = {}


def kernel(**inputs):
    last_err = None
    for nbin, cb in ((NBIN, CB), (53, 4), (51, 5), (55, 5)):
        try:
            in_maps, perms = prep_inputs(**inputs, nbin=nbin, cb=cb)
            break
        except AssertionError as e:
            last_err = e
    else:
        raise last_err
    key = (nbin, cb)
    if key not in _NC_CACHE:
        _NC_CACHE[key] = build_kernel(nbin, cb)
    nc = _NC_CACHE[key]
    res = run_bass_kernel_spmd(nc, in_maps, list(range(NCORES)))
    result = np.empty((N, HC), np.float32)
    for core in range(NCORES):
        o = res.results[core]["out"]
        p = perms[core]
        valid = p >= 0
        result[p[valid]] = o[valid]
    return result
